# revision 15
# baseline (speedup 1.0000x reference)
"""GAT message-passing kernel for TRN2: host preprocessing + Bass/Tile program.

v4 "alpha-field" design (per core, SPMD over 8 cores, dst-sharded nodes):
  phase 0: EVERY core computes the FULL feat table (x @ W_gat, bf16) from a
           host-rotated copy of x so its own dst shard occupies blocks 0..48.
           No collective. er = feat . attn_r for own blocks -> er_flat row.
  edge phase, per dst block of 128 nodes (edges pre-sorted by (dst blk, src)):
    - erb = ones x er_row rank-1 matmul  (PSUM [P,P]: er[d] bcast down parts)
    - dma_gather feat rows of the block's edge sources (bf16 256B rows,
      lo/hi half-table gathers for int16 indices)   <- gpsimd critical path
    - el (per edge) batched: t = G*attn_l (bf16), el = reduce_X(t)  [2 DVE]
    - per 128-edge chunk j:
        lz  = ACT Lrelu(erb + el_col_j)      (alpha-field, [P,128] bf16)
        alf = ACT Exp(lz)
        Oa  = DVE stt: (iota == dst_j) * alf (one DVE op per chunk)
        pA += Oa^T @ G_j      (agg unnorm, PSUM [128 dst, hid])
        pB += Oa^T @ ones_col (denom,      PSUM [128 dst, 1])
    - tail: rec = 1/max(pB,eps) [P,1]; gene = pA*rec + bias;
      geneL = ACT Lrelu(0.01) bf16; out = (geneL^T)^T @ W_lin^T via PE
      transpose + bf16 matmul.
Softmax max-subtraction dropped (exp args bounded ~ +-13; ratios identical).
"""

import numpy as np
import ml_dtypes
from contextlib import ExitStack

import concourse.bass as bass
import concourse.tile as tile
from concourse import bacc, mybir
from concourse import library_config

dt = mybir.dt
P = 128
PAD_DST = 512.0  # one-hot miss sentinel (exact in bf16, > 127)


# ---------------------------------------------------------------- host side

def preprocess(src, dst, n_nodes, n_cores):
    """Pure index-space preprocessing (no float math on values).

    Per-core node rotation: core c's table is the global padded table rotated
    by c*npc_pad so that its own dst shard occupies rows [0, npc_pad).
    """
    src = np.asarray(src).astype(np.int64)
    dst = np.asarray(dst).astype(np.int64)
    npc = n_nodes // n_cores                      # nodes per core
    assert npc * n_cores == n_nodes
    blocks = (npc + P - 1) // P
    npc_pad = blocks * P                          # padded nodes per core
    n_pad = npc_pad * n_cores                     # padded global node count
    half = n_pad // 2                             # low table rows [0, half)
    assert half <= 32767 and (n_pad - half) <= 32767

    core_of = dst // npc
    blk_of = (dst % npc) // P
    dloc_of = (dst % npc) % P
    srcp = (src // npc) * npc_pad + (src % npc)   # padded source coordinates

    order = np.lexsort((srcp, blk_of, core_of))
    s_s, c_s, b_s, d_s = srcp[order], core_of[order], blk_of[order], dloc_of[order]

    # per-core rotated source ids and lo/hi bucketing
    lo_lists = [[None] * blocks for _ in range(n_cores)]
    hi_lists = [[None] * blocks for _ in range(n_cores)]
    for c in range(n_cores):
        cm = c_s == c
        s_rot = (s_s - c * npc_pad) % n_pad
        hi_mask = s_rot >= half
        for b in range(blocks):
            m = cm & (b_s == b)
            ml = m & ~hi_mask
            mh = m & hi_mask
            ol = np.argsort(s_rot[ml], kind="stable")
            oh = np.argsort(s_rot[mh], kind="stable")
            lo_lists[c][b] = (s_rot[ml][ol], d_s[ml][ol])
            hi_lists[c][b] = (s_rot[mh][oh] - half, d_s[mh][oh])

    def nchunks(n):
        return (n + P - 1) // P

    C_lo = [max(max(nchunks(len(lo_lists[c][b][0])) for c in range(n_cores)), 1)
            for b in range(blocks)]
    C_hi = [max(nchunks(len(hi_lists[c][b][0])) for c in range(n_cores))
            for b in range(blocks)]

    total_chunks = sum(C_lo) + sum(C_hi)
    total_L = total_chunks * P

    per_core = []
    for c in range(n_cores):
        idx = np.zeros(total_L, dtype=np.int16)
        dstf = np.full(total_L, PAD_DST, dtype=np.float32)
        off = 0
        for b in range(blocks):
            for lists, C in ((lo_lists, C_lo[b]), (hi_lists, C_hi[b])):
                L = C * P
                if L == 0:
                    continue
                s_arr, d_arr = lists[c][b]
                n = len(s_arr)
                idx[off:off + n] = s_arr.astype(np.int16)
                # pads keep idx 0 (real row; killed by dst sentinel)
                dstf[off:off + n] = d_arr.astype(np.float32)
                off += L
        assert off == total_L
        # wrapped int16 layout: index i lives at [i % 16, i // 16];
        # replicated 8x down partitions (one copy per Q7 core)
        idx16 = np.tile(idx.reshape(total_L // 16, 16).T, (8, 1)).copy()
        # chunk-major dst columns: chunk g partition e = dstf[g*128+e]
        dstf2 = dstf.reshape(total_chunks, P).T.copy()
        per_core.append({"idx16": idx16, "dstf": dstf2})

    sched = {
        "n_nodes": n_nodes, "n_cores": n_cores, "npc": npc, "blocks": blocks,
        "npc_pad": npc_pad, "n_pad": n_pad,
        "half": half, "C_lo": C_lo, "C_hi": C_hi,
        "total_chunks": total_chunks, "total_L": total_L,
        "CMAX": max(C_lo[b] + C_hi[b] for b in range(blocks)),
    }
    return sched, per_core


def make_core_inputs(sched, per_core, x, W_gat, attn_l, attn_r, bias_gat, W_lin):
    """Per-core in_maps. Only permutation/replication/padding/dtype-cast."""
    n_cores, npc, blocks = sched["n_cores"], sched["npc"], sched["blocks"]
    npc_pad, n_pad = sched["npc_pad"], sched["n_pad"]
    in_f = x.shape[1]
    hid = W_gat.shape[1]
    bf = ml_dtypes.bfloat16
    x = np.asarray(x, dtype=np.float32)
    # padded node-major x (global): row (c*npc_pad + i) = x[c*npc + i]
    xpad = np.zeros((n_pad, in_f), dtype=np.float32)
    for c in range(n_cores):
        xpad[c * npc_pad:c * npc_pad + npc] = x[c * npc:(c + 1) * npc]
    Wg_bf = np.asarray(W_gat, np.float32).astype(bf)
    Wg_ext = np.zeros((in_f, hid + 2), dtype=bf)
    Wg_ext[:, 0:hid] = Wg_bf
    WgT_bf = np.ascontiguousarray(np.asarray(W_gat, np.float32).T).astype(bf)
    al_col = np.asarray(attn_l, np.float32).astype(bf)[:, None].copy()
    attnr_b = np.broadcast_to(np.asarray(attn_r, np.float32), (P, hid)).copy()
    bias_b = np.broadcast_to(np.asarray(bias_gat, np.float32), (P, hid)).copy()
    WlT_bf = np.ascontiguousarray(np.asarray(W_lin, np.float32).T).astype(bf)
    iota_bf = np.broadcast_to(np.arange(P, dtype=bf), (P, P)).copy()
    ident_bf = np.eye(P, dtype=bf)
    in_maps = []
    for c in range(n_cores):
        # rotate so core c's shard is first
        xrot = np.roll(xpad, -c * npc_pad, axis=0)
        m = {
            "xT": np.ascontiguousarray(xrot.T).astype(bf),  # [in_f, n_pad]
            "Wg": Wg_ext,
            "WgT": WgT_bf,
            "al_col": al_col,
            "attnr_b": attnr_b,
            "bias_b": bias_b,
            "WlT": WlT_bf,
            "iota_bf": iota_bf,
            "ident_bf": ident_bf,
            "idx16": per_core[c]["idx16"],
            "dstf": per_core[c]["dstf"],
        }
        in_maps.append(m)
    return in_maps


# ---------------------------------------------------------------- device side

def build_program(sched, in_f, hid, out_f, attn_slope=0.2, act_slope=0.01,
                  n_repeat=1):
    n_cores = sched["n_cores"]
    blocks, half, n_pad = sched["blocks"], sched["half"], sched["n_pad"]
    C_lo, C_hi = sched["C_lo"], sched["C_hi"]
    total_chunks, total_L = sched["total_chunks"], sched["total_L"]
    CMAX = sched["CMAX"]
    nblk_all = n_pad // P
    assert in_f % P == 0 and hid == P
    KT = in_f // P

    nc = bacc.Bacc("TRN2", target_bir_lowering=False, debug=False,
                   num_devices=n_cores, num_swdge_queues=4)

    def din(name, shape, dtype):
        return nc.dram_tensor(name, shape, dtype, kind="ExternalInput").ap()

    HE = 256  # extended table row: [feat(128) | 1.0 | el | junk]
    xT = din("xT", [in_f, n_pad], dt.bfloat16)
    Wg = din("Wg", [in_f, hid + 2], dt.bfloat16)
    WgT = din("WgT", [hid, in_f], dt.bfloat16)
    al_col_in = din("al_col", [hid, 1], dt.bfloat16)
    attnr_b = din("attnr_b", [P, hid], dt.float32)
    bias_b = din("bias_b", [P, hid], dt.float32)
    WlT = din("WlT", [hid, out_f], dt.bfloat16)
    iota_bf = din("iota_bf", [P, P], dt.bfloat16)
    ident_bf = din("ident_bf", [P, P], dt.bfloat16)
    idx16 = din("idx16", [128, total_L // 16], dt.int16)
    dstf = din("dstf", [P, total_chunks], dt.float32)
    out = nc.dram_tensor("out", [blocks * P, out_f], dt.float32,
                         kind="ExternalOutput").ap()

    table_lo = nc.dram_tensor("table_lo", [half, HE], dt.bfloat16).ap()
    table_hi = nc.dram_tensor("table_hi", [n_pad - half, HE], dt.bfloat16).ap()
    er_lin = nc.dram_tensor("er_lin", [blocks * P], dt.bfloat16).ap()

    BW = 7  # phase-0 node blocks per DMA batch (aligns half=28*BW*P)
    assert nblk_all % BW == 0

    with ExitStack() as ctx:
        tc = ctx.enter_context(tile.TileContext(nc))
        nc.gpsimd.load_library(library_config.mlp)
        const = ctx.enter_context(tc.tile_pool(name="const", bufs=1))

        # ---- constants in SBUF
        iota_sb = const.tile([P, P], dt.bfloat16)
        nc.sync.dma_start(iota_sb[:], iota_bf[:])
        ident_sb = const.tile([P, P], dt.bfloat16)
        nc.sync.dma_start(ident_sb[:], ident_bf[:])
        attnr_sb = const.tile([P, hid], dt.float32)
        nc.sync.dma_start(attnr_sb[:], attnr_b[:])
        bias_sb = const.tile([P, hid], dt.float32)
        nc.sync.dma_start(bias_sb[:], bias_b[:])
        WlT_sb = const.tile([hid, out_f], dt.bfloat16)
        nc.sync.dma_start(WlT_sb[:], WlT[:])
        WgT_sb = const.tile([hid, in_f], dt.bfloat16)
        nc.sync.dma_start(WgT_sb[:], WgT[:])
        al_col_sb = const.tile([hid, 1], dt.bfloat16)
        nc.sync.dma_start(al_col_sb[:], al_col_in[:])
        Wg_sb = []
        for k in range(KT):
            w = const.tile([P, hid + 2], dt.bfloat16, tag=f"Wg{k}")
            nc.sync.dma_start(w[:, 0:hid + 2], Wg[k * P:(k + 1) * P, :])
            Wg_sb.append(w)
        al02 = const.tile([P, 1], dt.float32)
        nc.vector.memset(al02[:], 0.2)
        al001 = const.tile([P, 1], dt.float32)
        nc.vector.memset(al001[:], 0.01)
        ones_row = const.tile([1, P], dt.bfloat16)
        nc.vector.memset(ones_row[:], 1.0)
        ones_col = const.tile([P, 1], dt.bfloat16)
        nc.vector.memset(ones_col[:], 1.0)
        idx_sb = const.tile([128, total_L // 16], dt.int16)
        nc.sync.dma_start(idx_sb[:], idx16[:])
        dst_sb = const.tile([P, total_chunks], dt.float32)
        nc.sync.dma_start(dst_sb[:], dstf[:])
        er_sb = const.tile([1, blocks * P], dt.bfloat16)

        # ---- phase 0: full feat table + er for own blocks
        xp = ctx.enter_context(tc.tile_pool(name="xp", bufs=6))
        ps0 = ctx.enter_context(tc.tile_pool(name="ps0", bufs=2, space="PSUM"))
        fbp = ctx.enter_context(tc.tile_pool(name="fbp", bufs=4))
        erp = ctx.enter_context(tc.tile_pool(name="erp", bufs=2))

        # v = Wg @ attn_l -> Wg_ext column hid+1 (el producer)
        for k in range(KT):
            psV = ps0.tile([P, 1], dt.float32, tag="fp")
            nc.tensor.matmul(psV[:], lhsT=WgT_sb[:, k * P:(k + 1) * P],
                             rhs=al_col_sb[:], start=True, stop=True)
            nc.scalar.copy(Wg_sb[k][:, hid + 1:hid + 2], psV[:])
        assert half % (BW * P) == 0
        for g4 in range(nblk_all // BW):
            r0 = g4 * BW * P
            tbl = table_lo if r0 < half else table_hi
            toff = r0 if r0 < half else r0 - half
            xa = []
            for k in range(KT):
                t = xp.tile([P, BW * P], dt.bfloat16, tag=f"x{k}")
                nc.sync.dma_start(t[:], xT[k * P:(k + 1) * P,
                                            r0:r0 + BW * P])
                xa.append(t)
            fb4 = fbp.tile([P, BW * HE], dt.bfloat16, tag="fb4")
            nc.vector.memset(
                fb4[:].rearrange("p (i h) -> p i h", h=HE)[:, :, hid:hid + 1],
                1.0)
            for i in range(BW):
                nb = g4 * BW + i
                fp = ps0.tile([P, hid + 2], dt.float32, tag="fp")
                for k in range(KT):
                    nc.tensor.matmul(fp[:], lhsT=xa[k][:, i * P:(i + 1) * P],
                                     rhs=Wg_sb[k][:], start=(k == 0),
                                     stop=(k == KT - 1))
                nc.scalar.copy(fb4[:, i * HE:i * HE + hid], fp[:, 0:hid])
                nc.scalar.copy(fb4[:, i * HE + hid + 1:i * HE + hid + 2],
                               fp[:, hid + 1:hid + 2])
                if nb < blocks:
                    er_f = erp.tile([P, 1], dt.float32, tag="erf")
                    scr = erp.tile([P, hid], dt.float32, tag="scr")
                    nc.vector.scalar_tensor_tensor(
                        out=scr[:], in0=fp[:, 0:hid], scalar=1.0,
                        in1=attnr_sb[:],
                        op0=mybir.AluOpType.bypass, op1=mybir.AluOpType.mult,
                        accum_out=er_f[:])
                    er_b = erp.tile([P, 1], dt.bfloat16, tag="erb16")
                    nc.scalar.copy(er_b[:], er_f[:])
                    nc.scalar.dma_start(er_lin[nb * P:(nb + 1) * P, None],
                                        er_b[:])
            nc.scalar.dma_start(
                tbl[toff:toff + BW * P, :].rearrange(
                    "(i p) h -> p i h", i=BW),
                fb4[:].rearrange("p (i h) -> p i h", h=HE))
        nc.sync.dma_start(er_sb[:], er_lin[:][None, :])

        # ---- edge phase
        gp = ctx.enter_context(tc.tile_pool(name="gp", bufs=3))
        zp = ctx.enter_context(tc.tile_pool(name="zp", bufs=2))
        lzp = ctx.enter_context(tc.tile_pool(name="lzp", bufs=2))
        alp = ctx.enter_context(tc.tile_pool(name="alp", bufs=2))
        oap = ctx.enter_context(tc.tile_pool(name="oap", bufs=3))
        tlp = ctx.enter_context(tc.tile_pool(name="tlp", bufs=2))
        psErb = ctx.enter_context(tc.tile_pool(name="psErb", bufs=2, space="PSUM"))
        psA = ctx.enter_context(tc.tile_pool(name="psA", bufs=2, space="PSUM"))
        psT = ctx.enter_context(tc.tile_pool(name="psT", bufs=1, space="PSUM"))

        _gq = [0]
        loop_ctx = tc.For_i(0, n_repeat, 1) if n_repeat > 1 else None
        if loop_ctx is not None:
            loop_ctx.__enter__()
        if True:
          g = 0
          for b in range(blocks):
            Cl, Ch = C_lo[b], C_hi[b]
            C = Cl + Ch
            erb_ps = psErb.tile([P, P], dt.float32, tag="erb")
            nc.tensor.matmul(erb_ps[:], lhsT=ones_row[:],
                             rhs=er_sb[:, b * P:(b + 1) * P],
                             start=True, stop=True)

            G = gp.tile([P, C * HE], dt.bfloat16, tag="G")
            G3 = G[:].rearrange("p (c h) -> p c h", h=HE)
            o16 = (g * P) // 16
            GMAX = 6  # chunks per dma_gather; 768 idxs < 1024-desc SWDGE ring
            for cbase, ccnt, tbl in (
                    [(c0, min(GMAX, Cl - c0), table_lo)
                     for c0 in range(0, Cl, GMAX)] +
                    [(Cl + c0, min(GMAX, Ch - c0), table_hi)
                     for c0 in range(0, Ch, GMAX)]):
                nc.gpsimd.dma_gather(
                    G3[:, cbase:cbase + ccnt, :], tbl,
                    idx_sb[:, o16 + cbase * 8:o16 + (cbase + ccnt) * 8],
                    ccnt * P, ccnt * P, HE, elem_step=HE,
                    queue_num=_gq[0] % 4)
                _gq[0] += 1

            z = zp.tile([P, C * P], dt.bfloat16, tag="z")
            nc.vector.scalar_tensor_tensor(
                out=z[:].rearrange("p (c f) -> p c f", f=P),
                in0=erb_ps[:].unsqueeze(1).broadcast_to([P, C, P]),
                scalar=1.0,
                in1=G3[:, :, hid + 1:hid + 2].broadcast_to([P, C, P]),
                op0=mybir.AluOpType.bypass, op1=mybir.AluOpType.add)
            lzt = lzp.tile([P, C * P], dt.bfloat16, tag="lz")
            nc.vector.scalar_tensor_tensor(
                out=lzt[:], in0=z[:], scalar=float(attn_slope),
                in1=z[:], op0=mybir.AluOpType.mult, op1=mybir.AluOpType.max)
            alf = alp.tile([P, C * P], dt.bfloat16, tag="alf")
            nc.scalar.activation(alf[:], lzt[:],
                                 mybir.ActivationFunctionType.Exp)
            pAB = psA.tile([P, hid + 1], dt.float32, tag="psA")
            pA = pAB[:, 0:hid]
            pB = pAB[:, hid:hid + 1]
            for j in range(C):
                Oa = oap.tile([P, P], dt.bfloat16, tag="oa")
                nc.vector.scalar_tensor_tensor(
                    out=Oa[:], in0=iota_sb[:], scalar=dst_sb[:, g + j:g + j + 1],
                    in1=alf[:, j * P:(j + 1) * P], op0=mybir.AluOpType.is_equal,
                    op1=mybir.AluOpType.mult)
                nc.tensor.matmul(pAB[:], lhsT=Oa[:], rhs=G3[:, j, 0:hid + 1],
                                 start=(j == 0), stop=(j == C - 1))
            g += C

            den = tlp.tile([P, 1], dt.float32, tag="den")
            nc.vector.tensor_scalar(den[:], pB, 1e-30, None,
                                    mybir.AluOpType.max)
            rec = tlp.tile([P, 1], dt.float32, tag="rec")
            nc.vector.reciprocal(rec[:], den[:])
            gene = tlp.tile([P, hid], dt.float32, tag="gene")
            nc.vector.scalar_tensor_tensor(
                out=gene[:], in0=pA, scalar=rec[:], in1=bias_sb[:],
                op0=mybir.AluOpType.mult, op1=mybir.AluOpType.add)
            geneL = tlp.tile([P, hid], dt.bfloat16, tag="geneL")
            nc.scalar.activation(geneL[:], gene[:],
                                 mybir.ActivationFunctionType.Prelu,
                                 alpha=al001[:])
            gT_ps = psT.tile([hid, P], dt.bfloat16, tag="gT")
            nc.tensor.transpose(gT_ps[:], geneL[:], ident_sb[:])
            gT = tlp.tile([hid, P], dt.bfloat16, tag="gTs")
            nc.scalar.copy(gT[:], gT_ps[:])
            o_ps = psT.tile([P, out_f], dt.float32, tag="ops")
            nc.tensor.matmul(o_ps[:], lhsT=gT[:], rhs=WlT_sb[:],
                             start=True, stop=True)
            o_sb = tlp.tile([P, out_f], dt.float32, tag="osb")
            nc.scalar.copy(o_sb[:], o_ps[:])
            nc.sync.dma_start(out[b * P:(b + 1) * P, :], o_sb[:])
        if loop_ctx is not None:
            loop_ctx.__exit__(None, None, None)

    nc.compile()
    return nc


# ---------------------------------------------------------------- entry point

N_NODES, N_EDGES, IN_F, HID, OUT_F = 50000, 800000, 256, 128, 64
N_CORES = 8

_cache = {}


def kernel(x, src, dst, W_gat, attn_l, attn_r, bias_gat, W_lin):
    """Full-input GAT layer on 8 NeuronCores; returns [N_NODES, OUT_F] fp32."""
    from concourse.bass_utils import run_bass_kernel_spmd

    src = np.asarray(src)
    dst = np.asarray(dst)
    key = (src.tobytes(), dst.tobytes())
    ck = _cache.get("k")
    if ck is not None and ck[0] == key:
        sched, nc = ck[1], ck[2]
    else:
        sched, per_core = preprocess(src, dst, N_NODES, N_CORES)
        _cache["pc"] = per_core
        nc = build_program(sched, IN_F, HID, OUT_F)
        _cache["k"] = (key, sched, nc)
        ck = _cache["k"]
    sched = ck[1]
    per_core = _cache["pc"]
    in_maps = make_core_inputs(sched, per_core, x, W_gat, attn_l, attn_r,
                               bias_gat, W_lin)
    res = run_bass_kernel_spmd(nc, in_maps, core_ids=list(range(N_CORES)))
    out = np.concatenate(
        [res.results[c]["out"][:sched["npc"]] for c in range(N_CORES)], axis=0)
    return out.astype(np.float32)


# revision 17
# speedup vs baseline: 1.1399x; 1.1399x over previous
"""GAT message-passing kernel for TRN2: host preprocessing + Bass/Tile program.

v4 "alpha-field" design (per core, SPMD over 8 cores, dst-sharded nodes):
  phase 0: EVERY core computes the FULL feat table (x @ W_gat, bf16) from a
           host-rotated copy of x so its own dst shard occupies blocks 0..48.
           No collective. er = feat . attn_r for own blocks -> er_flat row.
  edge phase, per dst block of 128 nodes (edges pre-sorted by (dst blk, src)):
    - erb = ones x er_row rank-1 matmul  (PSUM [P,P]: er[d] bcast down parts)
    - dma_gather feat rows of the block's edge sources (bf16 256B rows,
      lo/hi half-table gathers for int16 indices)   <- gpsimd critical path
    - el (per edge) batched: t = G*attn_l (bf16), el = reduce_X(t)  [2 DVE]
    - per 128-edge chunk j:
        lz  = ACT Lrelu(erb + el_col_j)      (alpha-field, [P,128] bf16)
        alf = ACT Exp(lz)
        Oa  = DVE stt: (iota == dst_j) * alf (one DVE op per chunk)
        pA += Oa^T @ G_j      (agg unnorm, PSUM [128 dst, hid])
        pB += Oa^T @ ones_col (denom,      PSUM [128 dst, 1])
    - tail: rec = 1/max(pB,eps) [P,1]; gene = pA*rec + bias;
      geneL = ACT Lrelu(0.01) bf16; out = (geneL^T)^T @ W_lin^T via PE
      transpose + bf16 matmul.
Softmax max-subtraction dropped (exp args bounded ~ +-13; ratios identical).
"""

import numpy as np
import ml_dtypes
from contextlib import ExitStack

import concourse.bass as bass
import concourse.tile as tile
from concourse import bacc, mybir
from concourse import library_config

dt = mybir.dt
P = 128
PAD_DST = 512.0  # one-hot miss sentinel (exact in bf16, > 127)


# ---------------------------------------------------------------- host side

def preprocess(src, dst, n_nodes, n_cores):
    """Pure index-space preprocessing (no float math on values).

    Per-core node rotation: core c's table is the global padded table rotated
    by c*npc_pad so that its own dst shard occupies rows [0, npc_pad).
    """
    src = np.asarray(src).astype(np.int64)
    dst = np.asarray(dst).astype(np.int64)
    npc = n_nodes // n_cores                      # nodes per core
    assert npc * n_cores == n_nodes
    blocks = (npc + P - 1) // P
    npc_pad = blocks * P                          # padded nodes per core
    n_pad = npc_pad * n_cores                     # padded global node count
    half = n_pad // 2                             # low table rows [0, half)
    assert half <= 32767 and (n_pad - half) <= 32767

    core_of = dst // npc
    blk_of = (dst % npc) // P
    dloc_of = (dst % npc) % P
    srcp = (src // npc) * npc_pad + (src % npc)   # padded source coordinates

    order = np.lexsort((srcp, blk_of, core_of))
    s_s, c_s, b_s, d_s = srcp[order], core_of[order], blk_of[order], dloc_of[order]

    # per-core rotated source ids and lo/hi bucketing
    lo_lists = [[None] * blocks for _ in range(n_cores)]
    hi_lists = [[None] * blocks for _ in range(n_cores)]
    for c in range(n_cores):
        cm = c_s == c
        s_rot = (s_s - c * npc_pad) % n_pad
        hi_mask = s_rot >= half
        for b in range(blocks):
            m = cm & (b_s == b)
            ml = m & ~hi_mask
            mh = m & hi_mask
            ol = np.argsort(s_rot[ml], kind="stable")
            oh = np.argsort(s_rot[mh], kind="stable")
            lo_lists[c][b] = (s_rot[ml][ol], d_s[ml][ol])
            hi_lists[c][b] = (s_rot[mh][oh] - half, d_s[mh][oh])

    def nchunks(n):
        return (n + P - 1) // P

    C_lo = [max(max(nchunks(len(lo_lists[c][b][0])) for c in range(n_cores)), 1)
            for b in range(blocks)]
    C_hi = [max(nchunks(len(hi_lists[c][b][0])) for c in range(n_cores))
            for b in range(blocks)]

    total_chunks = sum(C_lo) + sum(C_hi)
    total_L = total_chunks * P

    per_core = []
    for c in range(n_cores):
        idx = np.zeros(total_L, dtype=np.int16)
        dstf = np.full(total_L, PAD_DST, dtype=np.float32)
        off = 0
        for b in range(blocks):
            for lists, C in ((lo_lists, C_lo[b]), (hi_lists, C_hi[b])):
                L = C * P
                if L == 0:
                    continue
                s_arr, d_arr = lists[c][b]
                n = len(s_arr)
                idx[off:off + n] = s_arr.astype(np.int16)
                # pads keep idx 0 (real row; killed by dst sentinel)
                dstf[off:off + n] = d_arr.astype(np.float32)
                off += L
        assert off == total_L
        # wrapped int16 layout: index i lives at [i % 16, i // 16];
        # replicated 8x down partitions (one copy per Q7 core)
        idx16 = np.tile(idx.reshape(total_L // 16, 16).T, (8, 1)).copy()
        # additive alpha-field mask: 0 at (e, dst_e), -1000 elsewhere (leaky
        # scales negatives by 0.2 -> exp(0.2*(z-1000)) underflows to 0); pads all -1000
        dloc = dstf.reshape(total_chunks, P).astype(np.int64)  # [g, e]
        M3 = np.full((total_chunks, P, P), -1000.0, dtype=ml_dtypes.bfloat16)
        gg, ee = np.nonzero(dloc < P)
        M3[gg, ee, dloc[gg, ee]] = 0.0
        Mmask = np.ascontiguousarray(
            M3.transpose(1, 0, 2).reshape(P, total_chunks * P))
        per_core.append({"idx16": idx16, "Mmask": Mmask})

    sched = {
        "n_nodes": n_nodes, "n_cores": n_cores, "npc": npc, "blocks": blocks,
        "npc_pad": npc_pad, "n_pad": n_pad,
        "half": half, "C_lo": C_lo, "C_hi": C_hi,
        "total_chunks": total_chunks, "total_L": total_L,
        "CMAX": max(C_lo[b] + C_hi[b] for b in range(blocks)),
    }
    return sched, per_core


def make_core_inputs(sched, per_core, x, W_gat, attn_l, attn_r, bias_gat, W_lin):
    """Per-core in_maps. Only permutation/replication/padding/dtype-cast."""
    n_cores, npc, blocks = sched["n_cores"], sched["npc"], sched["blocks"]
    npc_pad, n_pad = sched["npc_pad"], sched["n_pad"]
    in_f = x.shape[1]
    hid = W_gat.shape[1]
    bf = ml_dtypes.bfloat16
    x = np.asarray(x, dtype=np.float32)
    # padded node-major x (global): row (c*npc_pad + i) = x[c*npc + i]
    xpad = np.zeros((n_pad, in_f), dtype=np.float32)
    for c in range(n_cores):
        xpad[c * npc_pad:c * npc_pad + npc] = x[c * npc:(c + 1) * npc]
    Wg_bf = np.asarray(W_gat, np.float32).astype(bf)
    Wg_ext = np.zeros((in_f, hid + 2), dtype=bf)
    Wg_ext[:, 0:hid] = Wg_bf
    WgT_bf = np.ascontiguousarray(np.asarray(W_gat, np.float32).T).astype(bf)
    al_col = np.asarray(attn_l, np.float32).astype(bf)[:, None].copy()
    attnr_b = np.broadcast_to(np.asarray(attn_r, np.float32), (P, hid)).copy()
    bias_b = np.broadcast_to(np.asarray(bias_gat, np.float32), (P, hid)).copy()
    WlT_bf = np.ascontiguousarray(np.asarray(W_lin, np.float32).T).astype(bf)
    ident_bf = np.eye(P, dtype=bf)
    in_maps = []
    for c in range(n_cores):
        # rotate so core c's shard is first
        xrot = np.roll(xpad, -c * npc_pad, axis=0)
        m = {
            "xT": np.ascontiguousarray(xrot.T).astype(bf),  # [in_f, n_pad]
            "Wg": Wg_ext,
            "WgT": WgT_bf,
            "al_col": al_col,
            "attnr_b": attnr_b,
            "bias_b": bias_b,
            "WlT": WlT_bf,
            "ident_bf": ident_bf,
            "idx16": per_core[c]["idx16"],
            "Mmask": per_core[c]["Mmask"],
        }
        in_maps.append(m)
    return in_maps


# ---------------------------------------------------------------- device side

def build_program(sched, in_f, hid, out_f, attn_slope=0.2, act_slope=0.01,
                  n_repeat=1):
    n_cores = sched["n_cores"]
    blocks, half, n_pad = sched["blocks"], sched["half"], sched["n_pad"]
    C_lo, C_hi = sched["C_lo"], sched["C_hi"]
    total_chunks, total_L = sched["total_chunks"], sched["total_L"]
    CMAX = sched["CMAX"]
    nblk_all = n_pad // P
    assert in_f % P == 0 and hid == P
    KT = in_f // P

    nc = bacc.Bacc("TRN2", target_bir_lowering=False, debug=False,
                   num_devices=n_cores, num_swdge_queues=4)

    def din(name, shape, dtype):
        return nc.dram_tensor(name, shape, dtype, kind="ExternalInput").ap()

    HE = 256  # extended table row: [feat(128) | 1.0 | el | junk]
    xT = din("xT", [in_f, n_pad], dt.bfloat16)
    Wg = din("Wg", [in_f, hid + 2], dt.bfloat16)
    WgT = din("WgT", [hid, in_f], dt.bfloat16)
    al_col_in = din("al_col", [hid, 1], dt.bfloat16)
    attnr_b = din("attnr_b", [P, hid], dt.float32)
    bias_b = din("bias_b", [P, hid], dt.float32)
    WlT = din("WlT", [hid, out_f], dt.bfloat16)
    ident_bf = din("ident_bf", [P, P], dt.bfloat16)
    idx16 = din("idx16", [128, total_L // 16], dt.int16)
    Mmask = din("Mmask", [P, total_L], dt.bfloat16)
    out = nc.dram_tensor("out", [blocks * P, out_f], dt.float32,
                         kind="ExternalOutput").ap()

    table_lo = nc.dram_tensor("table_lo", [half, HE], dt.bfloat16).ap()
    table_hi = nc.dram_tensor("table_hi", [n_pad - half, HE], dt.bfloat16).ap()
    er_lin = nc.dram_tensor("er_lin", [blocks * P], dt.bfloat16).ap()

    BW = 7  # phase-0 node blocks per DMA batch (aligns half=28*BW*P)
    assert nblk_all % BW == 0

    with ExitStack() as ctx:
        tc = ctx.enter_context(tile.TileContext(nc))
        nc.gpsimd.load_library(library_config.mlp)
        const = ctx.enter_context(tc.tile_pool(name="const", bufs=1))

        # ---- constants in SBUF
        ident_sb = const.tile([P, P], dt.bfloat16)
        nc.sync.dma_start(ident_sb[:], ident_bf[:])
        attnr_sb = const.tile([P, hid], dt.float32)
        nc.sync.dma_start(attnr_sb[:], attnr_b[:])
        bias_sb = const.tile([P, hid], dt.float32)
        nc.sync.dma_start(bias_sb[:], bias_b[:])
        WlT_sb = const.tile([hid, out_f], dt.bfloat16)
        nc.sync.dma_start(WlT_sb[:], WlT[:])
        WgT_sb = const.tile([hid, in_f], dt.bfloat16)
        nc.sync.dma_start(WgT_sb[:], WgT[:])
        al_col_sb = const.tile([hid, 1], dt.bfloat16)
        nc.sync.dma_start(al_col_sb[:], al_col_in[:])
        Wg_sb = []
        for k in range(KT):
            w = const.tile([P, hid + 2], dt.bfloat16, tag=f"Wg{k}")
            nc.sync.dma_start(w[:, 0:hid + 2], Wg[k * P:(k + 1) * P, :])
            Wg_sb.append(w)
        al02 = const.tile([P, 1], dt.float32)
        nc.vector.memset(al02[:], 0.2)
        al001 = const.tile([P, 1], dt.float32)
        nc.vector.memset(al001[:], 0.01)
        ones_row = const.tile([1, P], dt.bfloat16)
        nc.vector.memset(ones_row[:], 1.0)
        ones_col = const.tile([P, 1], dt.bfloat16)
        nc.vector.memset(ones_col[:], 1.0)
        idx_sb = const.tile([128, total_L // 16], dt.int16)
        nc.sync.dma_start(idx_sb[:], idx16[:])
        er_sb = const.tile([1, blocks * P], dt.bfloat16)

        # ---- phase 0: full feat table + er for own blocks
        xp = ctx.enter_context(tc.tile_pool(name="xp", bufs=6))
        ps0 = ctx.enter_context(tc.tile_pool(name="ps0", bufs=2, space="PSUM"))
        fbp = ctx.enter_context(tc.tile_pool(name="fbp", bufs=4))
        erp = ctx.enter_context(tc.tile_pool(name="erp", bufs=2))

        # v = Wg @ attn_l -> Wg_ext column hid+1 (el producer)
        for k in range(KT):
            psV = ps0.tile([P, 1], dt.float32, tag="fp")
            nc.tensor.matmul(psV[:], lhsT=WgT_sb[:, k * P:(k + 1) * P],
                             rhs=al_col_sb[:], start=True, stop=True)
            nc.scalar.copy(Wg_sb[k][:, hid + 1:hid + 2], psV[:])
        assert half % (BW * P) == 0
        for g4 in range(nblk_all // BW):
            r0 = g4 * BW * P
            tbl = table_lo if r0 < half else table_hi
            toff = r0 if r0 < half else r0 - half
            xa = []
            for k in range(KT):
                t = xp.tile([P, BW * P], dt.bfloat16, tag=f"x{k}")
                nc.sync.dma_start(t[:], xT[k * P:(k + 1) * P,
                                            r0:r0 + BW * P])
                xa.append(t)
            fb4 = fbp.tile([P, BW * HE], dt.bfloat16, tag="fb4")
            nc.vector.memset(
                fb4[:].rearrange("p (i h) -> p i h", h=HE)[:, :, hid:hid + 1],
                1.0)
            for i in range(BW):
                nb = g4 * BW + i
                fp = ps0.tile([P, hid + 2], dt.float32, tag="fp")
                for k in range(KT):
                    nc.tensor.matmul(fp[:], lhsT=xa[k][:, i * P:(i + 1) * P],
                                     rhs=Wg_sb[k][:], start=(k == 0),
                                     stop=(k == KT - 1))
                if i % 2 == 0:
                    nc.scalar.copy(fb4[:, i * HE:i * HE + hid], fp[:, 0:hid])
                else:
                    nc.vector.tensor_copy(fb4[:, i * HE:i * HE + hid],
                                          fp[:, 0:hid])
                nc.scalar.copy(fb4[:, i * HE + hid + 1:i * HE + hid + 2],
                               fp[:, hid + 1:hid + 2])
                if nb < blocks:
                    er_f = erp.tile([P, 1], dt.float32, tag="erf")
                    scr = erp.tile([P, hid], dt.float32, tag="scr")
                    nc.vector.scalar_tensor_tensor(
                        out=scr[:], in0=fp[:, 0:hid], scalar=1.0,
                        in1=attnr_sb[:],
                        op0=mybir.AluOpType.bypass, op1=mybir.AluOpType.mult,
                        accum_out=er_f[:])
                    er_b = erp.tile([P, 1], dt.bfloat16, tag="erb16")
                    nc.scalar.copy(er_b[:], er_f[:])
                    nc.scalar.dma_start(er_lin[nb * P:(nb + 1) * P, None],
                                        er_b[:])
            nc.scalar.dma_start(
                tbl[toff:toff + BW * P, :].rearrange(
                    "(i p) h -> p i h", i=BW),
                fb4[:].rearrange("p (i h) -> p i h", h=HE))
        nc.sync.dma_start(er_sb[:], er_lin[:][None, :])

        # ---- edge phase
        gp = ctx.enter_context(tc.tile_pool(name="gp", bufs=3))
        mp = ctx.enter_context(tc.tile_pool(name="mp", bufs=3))
        ebp = ctx.enter_context(tc.tile_pool(name="ebp", bufs=2))
        zp = ctx.enter_context(tc.tile_pool(name="zp", bufs=2))
        alp = ctx.enter_context(tc.tile_pool(name="alp", bufs=3))
        tlp = ctx.enter_context(tc.tile_pool(name="tlp", bufs=2))
        psErb = ctx.enter_context(tc.tile_pool(name="psErb", bufs=2, space="PSUM"))
        psA = ctx.enter_context(tc.tile_pool(name="psA", bufs=2, space="PSUM"))
        psT = ctx.enter_context(tc.tile_pool(name="psT", bufs=1, space="PSUM"))

        _gq = [0]
        loop_ctx = tc.For_i(0, n_repeat, 1) if n_repeat > 1 else None
        if loop_ctx is not None:
            loop_ctx.__enter__()
        if True:
          g = 0
          for b in range(blocks):
            Cl, Ch = C_lo[b], C_hi[b]
            C = Cl + Ch
            erb_ps = psErb.tile([P, P], dt.float32, tag="erb")
            nc.tensor.matmul(erb_ps[:], lhsT=ones_row[:],
                             rhs=er_sb[:, b * P:(b + 1) * P],
                             start=True, stop=True)
            erb_sb = ebp.tile([P, P], dt.bfloat16, tag="erbs")
            nc.scalar.copy(erb_sb[:], erb_ps[:])
            Mt = mp.tile([P, C * P], dt.bfloat16, tag="Mt")
            nc.sync.dma_start(Mt[:], Mmask[:, g * P:(g + C) * P])

            G = gp.tile([P, C * HE], dt.bfloat16, tag="G")
            G3 = G[:].rearrange("p (c h) -> p c h", h=HE)
            o16 = (g * P) // 16
            GMAX = 6  # chunks per dma_gather; 768 idxs < 1024-desc SWDGE ring
            for cbase, ccnt, tbl in (
                    [(c0, min(GMAX, Cl - c0), table_lo)
                     for c0 in range(0, Cl, GMAX)] +
                    [(Cl + c0, min(GMAX, Ch - c0), table_hi)
                     for c0 in range(0, Ch, GMAX)]):
                nc.gpsimd.dma_gather(
                    G3[:, cbase:cbase + ccnt, :], tbl,
                    idx_sb[:, o16 + cbase * 8:o16 + (cbase + ccnt) * 8],
                    ccnt * P, ccnt * P, HE, elem_step=HE,
                    queue_num=_gq[0] % 4)
                _gq[0] += 1

            z1 = zp.tile([P, C * P], dt.bfloat16, tag="z1")
            nc.vector.scalar_tensor_tensor(
                out=z1[:].rearrange("p (c f) -> p c f", f=P),
                in0=Mt[:].rearrange("p (c f) -> p c f", f=P),
                scalar=1.0,
                in1=erb_sb[:].unsqueeze(1).broadcast_to([P, C, P]),
                op0=mybir.AluOpType.bypass, op1=mybir.AluOpType.add)
            z2 = zp.tile([P, C * P], dt.bfloat16, tag="z2")
            nc.vector.scalar_tensor_tensor(
                out=z2[:].rearrange("p (c f) -> p c f", f=P),
                in0=z1[:].rearrange("p (c f) -> p c f", f=P),
                scalar=1.0,
                in1=G3[:, :, hid + 1:hid + 2].broadcast_to([P, C, P]),
                op0=mybir.AluOpType.bypass, op1=mybir.AluOpType.add)
            lzt = zp.tile([P, C * P], dt.bfloat16, tag="lz")
            nc.vector.scalar_tensor_tensor(
                out=lzt[:], in0=z2[:], scalar=float(attn_slope),
                in1=z2[:], op0=mybir.AluOpType.mult, op1=mybir.AluOpType.max)
            alf = alp.tile([P, C * P], dt.bfloat16, tag="alf")
            nc.scalar.activation(alf[:], lzt[:],
                                 mybir.ActivationFunctionType.Exp)
            pAB = psA.tile([P, hid + 1], dt.float32, tag="psA")
            pA = pAB[:, 0:hid]
            pB = pAB[:, hid:hid + 1]
            for j in range(C):
                nc.tensor.matmul(pAB[:], lhsT=alf[:, j * P:(j + 1) * P],
                                 rhs=G3[:, j, 0:hid + 1],
                                 start=(j == 0), stop=(j == C - 1))
            g += C

            den = tlp.tile([P, 1], dt.float32, tag="den")
            nc.vector.tensor_scalar(den[:], pB, 1e-30, None,
                                    mybir.AluOpType.max)
            rec = tlp.tile([P, 1], dt.float32, tag="rec")
            nc.vector.reciprocal(rec[:], den[:])
            gene = tlp.tile([P, hid], dt.float32, tag="gene")
            nc.vector.scalar_tensor_tensor(
                out=gene[:], in0=pA, scalar=rec[:], in1=bias_sb[:],
                op0=mybir.AluOpType.mult, op1=mybir.AluOpType.add)
            geneL = tlp.tile([P, hid], dt.bfloat16, tag="geneL")
            nc.scalar.activation(geneL[:], gene[:],
                                 mybir.ActivationFunctionType.Prelu,
                                 alpha=al001[:])
            gT_ps = psT.tile([hid, P], dt.bfloat16, tag="gT")
            nc.tensor.transpose(gT_ps[:], geneL[:], ident_sb[:])
            gT = tlp.tile([hid, P], dt.bfloat16, tag="gTs")
            nc.scalar.copy(gT[:], gT_ps[:])
            o_ps = psT.tile([P, out_f], dt.float32, tag="ops")
            nc.tensor.matmul(o_ps[:], lhsT=gT[:], rhs=WlT_sb[:],
                             start=True, stop=True)
            o_sb = tlp.tile([P, out_f], dt.float32, tag="osb")
            nc.scalar.copy(o_sb[:], o_ps[:])
            nc.sync.dma_start(out[b * P:(b + 1) * P, :], o_sb[:])
        if loop_ctx is not None:
            loop_ctx.__exit__(None, None, None)

    nc.compile()
    return nc


# ---------------------------------------------------------------- entry point

N_NODES, N_EDGES, IN_F, HID, OUT_F = 50000, 800000, 256, 128, 64
N_CORES = 8

_cache = {}


def kernel(x, src, dst, W_gat, attn_l, attn_r, bias_gat, W_lin):
    """Full-input GAT layer on 8 NeuronCores; returns [N_NODES, OUT_F] fp32."""
    from concourse.bass_utils import run_bass_kernel_spmd

    src = np.asarray(src)
    dst = np.asarray(dst)
    key = (src.tobytes(), dst.tobytes())
    ck = _cache.get("k")
    if ck is not None and ck[0] == key:
        sched, nc = ck[1], ck[2]
    else:
        sched, per_core = preprocess(src, dst, N_NODES, N_CORES)
        _cache["pc"] = per_core
        nc = build_program(sched, IN_F, HID, OUT_F)
        _cache["k"] = (key, sched, nc)
        ck = _cache["k"]
    sched = ck[1]
    per_core = _cache["pc"]
    in_maps = make_core_inputs(sched, per_core, x, W_gat, attn_l, attn_r,
                               bias_gat, W_lin)
    res = run_bass_kernel_spmd(nc, in_maps, core_ids=list(range(N_CORES)))
    out = np.concatenate(
        [res.results[c]["out"][:sched["npc"]] for c in range(N_CORES)], axis=0)
    return out.astype(np.float32)


# revision 18
# speedup vs baseline: 1.1955x; 1.0487x over previous
"""GAT message-passing kernel for TRN2: host preprocessing + Bass/Tile program.

v4 "alpha-field" design (per core, SPMD over 8 cores, dst-sharded nodes):
  phase 0: EVERY core computes the FULL feat table (x @ W_gat, bf16) from a
           host-rotated copy of x so its own dst shard occupies blocks 0..48.
           No collective. er = feat . attn_r for own blocks -> er_flat row.
  edge phase, per dst block of 128 nodes (edges pre-sorted by (dst blk, src)):
    - erb = ones x er_row rank-1 matmul  (PSUM [P,P]: er[d] bcast down parts)
    - dma_gather feat rows of the block's edge sources (bf16 256B rows,
      lo/hi half-table gathers for int16 indices)   <- gpsimd critical path
    - el (per edge) batched: t = G*attn_l (bf16), el = reduce_X(t)  [2 DVE]
    - per 128-edge chunk j:
        lz  = ACT Lrelu(erb + el_col_j)      (alpha-field, [P,128] bf16)
        alf = ACT Exp(lz)
        Oa  = DVE stt: (iota == dst_j) * alf (one DVE op per chunk)
        pA += Oa^T @ G_j      (agg unnorm, PSUM [128 dst, hid])
        pB += Oa^T @ ones_col (denom,      PSUM [128 dst, 1])
    - tail: rec = 1/max(pB,eps) [P,1]; gene = pA*rec + bias;
      geneL = ACT Lrelu(0.01) bf16; out = (geneL^T)^T @ W_lin^T via PE
      transpose + bf16 matmul.
Softmax max-subtraction dropped (exp args bounded ~ +-13; ratios identical).
"""

import numpy as np
import ml_dtypes
from contextlib import ExitStack

import concourse.bass as bass
import concourse.tile as tile
from concourse import bacc, mybir
from concourse import library_config

dt = mybir.dt
P = 128
PAD_DST = 512.0  # one-hot miss sentinel (exact in bf16, > 127)


# ---------------------------------------------------------------- host side

def preprocess(src, dst, n_nodes, n_cores):
    """Pure index-space preprocessing (no float math on values).

    Per-core node rotation: core c's table is the global padded table rotated
    by c*npc_pad so that its own dst shard occupies rows [0, npc_pad).
    """
    src = np.asarray(src).astype(np.int64)
    dst = np.asarray(dst).astype(np.int64)
    npc = n_nodes // n_cores                      # nodes per core
    assert npc * n_cores == n_nodes
    blocks = (npc + P - 1) // P
    npc_pad = blocks * P                          # padded nodes per core
    n_pad = npc_pad * n_cores                     # padded global node count
    half = n_pad // 2                             # low table rows [0, half)
    assert half <= 32767 and (n_pad - half) <= 32767

    core_of = dst // npc
    blk_of = (dst % npc) // P
    dloc_of = (dst % npc) % P
    srcp = (src // npc) * npc_pad + (src % npc)   # padded source coordinates

    order = np.lexsort((srcp, blk_of, core_of))
    s_s, c_s, b_s, d_s = srcp[order], core_of[order], blk_of[order], dloc_of[order]

    # per-core rotated source ids and lo/hi bucketing
    lo_lists = [[None] * blocks for _ in range(n_cores)]
    hi_lists = [[None] * blocks for _ in range(n_cores)]
    for c in range(n_cores):
        cm = c_s == c
        s_rot = (s_s - c * npc_pad) % n_pad
        hi_mask = s_rot >= half
        for b in range(blocks):
            m = cm & (b_s == b)
            ml = m & ~hi_mask
            mh = m & hi_mask
            ol = np.argsort(s_rot[ml], kind="stable")
            oh = np.argsort(s_rot[mh], kind="stable")
            lo_lists[c][b] = (s_rot[ml][ol], d_s[ml][ol])
            hi_lists[c][b] = (s_rot[mh][oh] - half, d_s[mh][oh])

    def nchunks(n):
        return (n + P - 1) // P

    C_lo = [max(max(nchunks(len(lo_lists[c][b][0])) for c in range(n_cores)), 1)
            for b in range(blocks)]
    C_hi = [max(nchunks(len(hi_lists[c][b][0])) for c in range(n_cores))
            for b in range(blocks)]

    total_chunks = sum(C_lo) + sum(C_hi)
    total_L = total_chunks * P

    per_core = []
    for c in range(n_cores):
        idx = np.zeros(total_L, dtype=np.int16)
        dstf = np.full(total_L, PAD_DST, dtype=np.float32)
        off = 0
        for b in range(blocks):
            for lists, C in ((lo_lists, C_lo[b]), (hi_lists, C_hi[b])):
                L = C * P
                if L == 0:
                    continue
                s_arr, d_arr = lists[c][b]
                n = len(s_arr)
                idx[off:off + n] = s_arr.astype(np.int16)
                # pads keep idx 0 (real row; killed by dst sentinel)
                dstf[off:off + n] = d_arr.astype(np.float32)
                off += L
        assert off == total_L
        # wrapped int16 layout: index i lives at [i % 16, i // 16];
        # replicated 8x down partitions (one copy per Q7 core)
        idx16 = np.tile(idx.reshape(total_L // 16, 16).T, (8, 1)).copy()
        # additive alpha-field mask: 0 at (e, dst_e), -1000 elsewhere (leaky
        # scales negatives by 0.2 -> exp(0.2*(z-1000)) underflows to 0); pads all -1000
        dloc = dstf.reshape(total_chunks, P).astype(np.int64)  # [g, e]
        M3 = np.full((total_chunks, P, P), -1000.0, dtype=ml_dtypes.bfloat16)
        gg, ee = np.nonzero(dloc < P)
        M3[gg, ee, dloc[gg, ee]] = 0.0
        Mmask = np.ascontiguousarray(
            M3.transpose(1, 0, 2).reshape(P, total_chunks * P))
        per_core.append({"idx16": idx16, "Mmask": Mmask})

    sched = {
        "n_nodes": n_nodes, "n_cores": n_cores, "npc": npc, "blocks": blocks,
        "npc_pad": npc_pad, "n_pad": n_pad,
        "half": half, "C_lo": C_lo, "C_hi": C_hi,
        "total_chunks": total_chunks, "total_L": total_L,
        "CMAX": max(C_lo[b] + C_hi[b] for b in range(blocks)),
    }
    return sched, per_core


def make_core_inputs(sched, per_core, x, W_gat, attn_l, attn_r, bias_gat, W_lin):
    """Per-core in_maps. Only permutation/replication/padding/dtype-cast."""
    n_cores, npc, blocks = sched["n_cores"], sched["npc"], sched["blocks"]
    npc_pad, n_pad = sched["npc_pad"], sched["n_pad"]
    in_f = x.shape[1]
    hid = W_gat.shape[1]
    bf = ml_dtypes.bfloat16
    x = np.asarray(x, dtype=np.float32)
    # padded node-major x (global): row (c*npc_pad + i) = x[c*npc + i]
    xpad = np.zeros((n_pad, in_f), dtype=np.float32)
    for c in range(n_cores):
        xpad[c * npc_pad:c * npc_pad + npc] = x[c * npc:(c + 1) * npc]
    Wg_bf = np.asarray(W_gat, np.float32).astype(bf)
    Wg_ext = np.zeros((in_f, hid + 2), dtype=bf)
    Wg_ext[:, 0:hid] = Wg_bf
    WgT_bf = np.ascontiguousarray(np.asarray(W_gat, np.float32).T).astype(bf)
    al_col = np.asarray(attn_l, np.float32).astype(bf)[:, None].copy()
    attnr_b = np.broadcast_to(np.asarray(attn_r, np.float32), (P, hid)).copy()
    bias_b = np.broadcast_to(np.asarray(bias_gat, np.float32), (P, hid)).copy()
    WlT_bf = np.ascontiguousarray(np.asarray(W_lin, np.float32).T).astype(bf)
    ident_bf = np.eye(P, dtype=bf)
    in_maps = []
    for c in range(n_cores):
        # rotate so core c's shard is first
        xrot = np.roll(xpad, -c * npc_pad, axis=0)
        m = {
            "xT": np.ascontiguousarray(xrot.T).astype(bf),  # [in_f, n_pad]
            "Wg": Wg_ext,
            "WgT": WgT_bf,
            "al_col": al_col,
            "attnr_b": attnr_b,
            "bias_b": bias_b,
            "WlT": WlT_bf,
            "ident_bf": ident_bf,
            "idx16": per_core[c]["idx16"],
            "Mmask": per_core[c]["Mmask"],
        }
        in_maps.append(m)
    return in_maps


# ---------------------------------------------------------------- device side

def build_program(sched, in_f, hid, out_f, attn_slope=0.2, act_slope=0.01,
                  n_repeat=1):
    n_cores = sched["n_cores"]
    blocks, half, n_pad = sched["blocks"], sched["half"], sched["n_pad"]
    C_lo, C_hi = sched["C_lo"], sched["C_hi"]
    total_chunks, total_L = sched["total_chunks"], sched["total_L"]
    CMAX = sched["CMAX"]
    nblk_all = n_pad // P
    assert in_f % P == 0 and hid == P
    KT = in_f // P

    nc = bacc.Bacc("TRN2", target_bir_lowering=False, debug=False,
                   num_devices=n_cores, num_swdge_queues=4)

    def din(name, shape, dtype):
        return nc.dram_tensor(name, shape, dtype, kind="ExternalInput").ap()

    HE = 256  # extended table row: [feat(128) | 1.0 | el | junk]
    xT = din("xT", [in_f, n_pad], dt.bfloat16)
    Wg = din("Wg", [in_f, hid + 2], dt.bfloat16)
    WgT = din("WgT", [hid, in_f], dt.bfloat16)
    al_col_in = din("al_col", [hid, 1], dt.bfloat16)
    attnr_b = din("attnr_b", [P, hid], dt.float32)
    bias_b = din("bias_b", [P, hid], dt.float32)
    WlT = din("WlT", [hid, out_f], dt.bfloat16)
    ident_bf = din("ident_bf", [P, P], dt.bfloat16)
    idx16 = din("idx16", [128, total_L // 16], dt.int16)
    Mmask = din("Mmask", [P, total_L], dt.bfloat16)
    out = nc.dram_tensor("out", [blocks * P, out_f], dt.float32,
                         kind="ExternalOutput").ap()

    table_lo = nc.dram_tensor("table_lo", [half, HE], dt.bfloat16).ap()
    table_hi = nc.dram_tensor("table_hi", [n_pad - half, HE], dt.bfloat16).ap()
    er_lin = nc.dram_tensor("er_lin", [blocks * P], dt.bfloat16).ap()

    BW = 7  # phase-0 node blocks per DMA batch (aligns half=28*BW*P)
    assert nblk_all % BW == 0

    with ExitStack() as ctx:
        tc = ctx.enter_context(tile.TileContext(nc))
        nc.gpsimd.load_library(library_config.mlp)
        const = ctx.enter_context(tc.tile_pool(name="const", bufs=1))

        # ---- constants in SBUF
        ident_sb = const.tile([P, P], dt.bfloat16)
        nc.sync.dma_start(ident_sb[:], ident_bf[:])
        attnr_sb = const.tile([P, hid], dt.float32)
        nc.sync.dma_start(attnr_sb[:], attnr_b[:])
        bias_sb = const.tile([P, hid], dt.float32)
        nc.sync.dma_start(bias_sb[:], bias_b[:])
        WlT_sb = const.tile([hid, out_f], dt.bfloat16)
        nc.sync.dma_start(WlT_sb[:], WlT[:])
        WgT_sb = const.tile([hid, in_f], dt.bfloat16)
        nc.sync.dma_start(WgT_sb[:], WgT[:])
        al_col_sb = const.tile([hid, 1], dt.bfloat16)
        nc.sync.dma_start(al_col_sb[:], al_col_in[:])
        Wg_sb = []
        for k in range(KT):
            w = const.tile([P, hid + 2], dt.bfloat16, tag=f"Wg{k}")
            nc.sync.dma_start(w[:, 0:hid + 2], Wg[k * P:(k + 1) * P, :])
            Wg_sb.append(w)
        al02 = const.tile([P, 1], dt.float32)
        nc.vector.memset(al02[:], 0.2)
        al001 = const.tile([P, 1], dt.float32)
        nc.vector.memset(al001[:], 0.01)
        ones_row = const.tile([1, P], dt.bfloat16)
        nc.vector.memset(ones_row[:], 1.0)
        ones_col = const.tile([P, 1], dt.bfloat16)
        nc.vector.memset(ones_col[:], 1.0)
        idx_sb = const.tile([128, total_L // 16], dt.int16)
        nc.sync.dma_start(idx_sb[:], idx16[:])
        er_sb = const.tile([1, blocks * P], dt.bfloat16)

        # ---- phase 0: full feat table + er for own blocks
        xp = ctx.enter_context(tc.tile_pool(name="xp", bufs=6))
        ps0 = ctx.enter_context(tc.tile_pool(name="ps0", bufs=3, space="PSUM"))
        fbp = ctx.enter_context(tc.tile_pool(name="fbp", bufs=4))
        erp = ctx.enter_context(tc.tile_pool(name="erp", bufs=2))

        # v = Wg @ attn_l -> Wg_ext column hid+1 (el producer)
        for k in range(KT):
            psV = ps0.tile([P, 1], dt.float32, tag="fp")
            nc.tensor.matmul(psV[:], lhsT=WgT_sb[:, k * P:(k + 1) * P],
                             rhs=al_col_sb[:], start=True, stop=True)
            nc.scalar.copy(Wg_sb[k][:, hid:hid + 1], psV[:])
        assert half % (BW * P) == 0
        for g4 in range(nblk_all // BW):
            r0 = g4 * BW * P
            tbl = table_lo if r0 < half else table_hi
            toff = r0 if r0 < half else r0 - half
            xa = xp.tile([P, KT * BW * P], dt.bfloat16, tag="xa")
            nc.sync.dma_start(
                xa[:].rearrange("p (k n) -> p k n", n=BW * P),
                xT[:, r0:r0 + BW * P].rearrange("(k p) n -> p k n", k=KT))
            fb4 = fbp.tile([P, BW * HE], dt.bfloat16, tag="fb4")
            nc.vector.memset(
                fb4[:].rearrange("p (i h) -> p i h", h=HE)[:, :,
                                                           hid + 1:hid + 2],
                1.0)
            for i in range(BW):
                nb = g4 * BW + i
                fp = ps0.tile([P, hid + 2], dt.float32, tag="fp")
                for k in range(KT):
                    nc.tensor.matmul(
                        fp[:], lhsT=xa[:, k * BW * P + i * P:
                                       k * BW * P + (i + 1) * P],
                        rhs=Wg_sb[k][:], start=(k == 0),
                        stop=(k == KT - 1))
                if i % 2 == 0:
                    nc.scalar.copy(fb4[:, i * HE:i * HE + hid + 1],
                                   fp[:, 0:hid + 1])
                else:
                    nc.vector.tensor_copy(fb4[:, i * HE:i * HE + hid + 1],
                                          fp[:, 0:hid + 1])
                if nb < blocks:
                    er_f = erp.tile([P, 1], dt.float32, tag="erf")
                    scr = erp.tile([P, hid], dt.float32, tag="scr")
                    nc.vector.scalar_tensor_tensor(
                        out=scr[:], in0=fp[:, 0:hid], scalar=1.0,
                        in1=attnr_sb[:],
                        op0=mybir.AluOpType.bypass, op1=mybir.AluOpType.mult,
                        accum_out=er_f[:])
                    er_b = erp.tile([P, 1], dt.bfloat16, tag="erb16")
                    nc.scalar.copy(er_b[:], er_f[:])
                    nc.scalar.dma_start(er_lin[nb * P:(nb + 1) * P, None],
                                        er_b[:])
            nc.scalar.dma_start(
                tbl[toff:toff + BW * P, :].rearrange(
                    "(i p) h -> p i h", i=BW),
                fb4[:].rearrange("p (i h) -> p i h", h=HE))
        nc.sync.dma_start(er_sb[:], er_lin[:][None, :])

        # ---- edge phase
        gp = ctx.enter_context(tc.tile_pool(name="gp", bufs=3))
        mp = ctx.enter_context(tc.tile_pool(name="mp", bufs=3))
        ebp = ctx.enter_context(tc.tile_pool(name="ebp", bufs=2))
        zp = ctx.enter_context(tc.tile_pool(name="zp", bufs=2))
        alp = ctx.enter_context(tc.tile_pool(name="alp", bufs=3))
        tlp = ctx.enter_context(tc.tile_pool(name="tlp", bufs=2))
        psErb = ctx.enter_context(tc.tile_pool(name="psErb", bufs=1, space="PSUM"))
        psA = ctx.enter_context(tc.tile_pool(name="psA", bufs=2, space="PSUM"))
        psT = ctx.enter_context(tc.tile_pool(name="psT", bufs=1, space="PSUM"))

        _gq = [0]
        loop_ctx = tc.For_i(0, n_repeat, 1) if n_repeat > 1 else None
        if loop_ctx is not None:
            loop_ctx.__enter__()
        if True:
          g = 0
          for b in range(blocks):
            Cl, Ch = C_lo[b], C_hi[b]
            C = Cl + Ch
            erb_ps = psErb.tile([P, P], dt.float32, tag="erb")
            nc.tensor.matmul(erb_ps[:], lhsT=ones_row[:],
                             rhs=er_sb[:, b * P:(b + 1) * P],
                             start=True, stop=True)
            erb_sb = ebp.tile([P, P], dt.bfloat16, tag="erbs")
            nc.scalar.copy(erb_sb[:], erb_ps[:])
            Mt = mp.tile([P, C * P], dt.bfloat16, tag="Mt")
            nc.sync.dma_start(Mt[:], Mmask[:, g * P:(g + C) * P])

            G = gp.tile([P, C * HE], dt.bfloat16, tag="G")
            G3 = G[:].rearrange("p (c h) -> p c h", h=HE)
            o16 = (g * P) // 16
            GMAX = 6  # chunks per dma_gather; 768 idxs < 1024-desc SWDGE ring
            for cbase, ccnt, tbl in (
                    [(c0, min(GMAX, Cl - c0), table_lo)
                     for c0 in range(0, Cl, GMAX)] +
                    [(Cl + c0, min(GMAX, Ch - c0), table_hi)
                     for c0 in range(0, Ch, GMAX)]):
                nc.gpsimd.dma_gather(
                    G3[:, cbase:cbase + ccnt, :], tbl,
                    idx_sb[:, o16 + cbase * 8:o16 + (cbase + ccnt) * 8],
                    ccnt * P, ccnt * P, HE, elem_step=HE,
                    queue_num=_gq[0] % 4)
                _gq[0] += 1

            z1 = zp.tile([P, C * P], dt.bfloat16, tag="z1")
            nc.vector.scalar_tensor_tensor(
                out=z1[:].rearrange("p (c f) -> p c f", f=P),
                in0=Mt[:].rearrange("p (c f) -> p c f", f=P),
                scalar=1.0,
                in1=erb_sb[:].unsqueeze(1).broadcast_to([P, C, P]),
                op0=mybir.AluOpType.bypass, op1=mybir.AluOpType.add)
            z2 = zp.tile([P, C * P], dt.bfloat16, tag="z2")
            nc.vector.scalar_tensor_tensor(
                out=z2[:].rearrange("p (c f) -> p c f", f=P),
                in0=z1[:].rearrange("p (c f) -> p c f", f=P),
                scalar=1.0,
                in1=G3[:, :, hid:hid + 1].broadcast_to([P, C, P]),
                op0=mybir.AluOpType.bypass, op1=mybir.AluOpType.add)
            lzt = zp.tile([P, C * P], dt.bfloat16, tag="lz")
            nc.vector.scalar_tensor_tensor(
                out=lzt[:], in0=z2[:], scalar=float(attn_slope),
                in1=z2[:], op0=mybir.AluOpType.mult, op1=mybir.AluOpType.max)
            alf = alp.tile([P, C * P], dt.bfloat16, tag="alf")
            nc.scalar.activation(alf[:], lzt[:],
                                 mybir.ActivationFunctionType.Exp)
            pAB = psA.tile([P, hid + 2], dt.float32, tag="psA")
            pA = pAB[:, 0:hid]
            pB = pAB[:, hid + 1:hid + 2]
            for j in range(C):
                nc.tensor.matmul(pAB[:], lhsT=alf[:, j * P:(j + 1) * P],
                                 rhs=G3[:, j, 0:hid + 2],
                                 start=(j == 0), stop=(j == C - 1))
            g += C

            den = tlp.tile([P, 1], dt.float32, tag="den")
            nc.vector.tensor_scalar(den[:], pB, 1e-30, None,
                                    mybir.AluOpType.max)
            rec = tlp.tile([P, 1], dt.float32, tag="rec")
            nc.vector.reciprocal(rec[:], den[:])
            gene = tlp.tile([P, hid], dt.float32, tag="gene")
            nc.vector.scalar_tensor_tensor(
                out=gene[:], in0=pA, scalar=rec[:], in1=bias_sb[:],
                op0=mybir.AluOpType.mult, op1=mybir.AluOpType.add)
            geneL = tlp.tile([P, hid], dt.bfloat16, tag="geneL")
            nc.scalar.activation(geneL[:], gene[:],
                                 mybir.ActivationFunctionType.Prelu,
                                 alpha=al001[:])
            gT_ps = psT.tile([hid, P], dt.bfloat16, tag="gT")
            nc.tensor.transpose(gT_ps[:], geneL[:], ident_sb[:])
            gT = tlp.tile([hid, P], dt.bfloat16, tag="gTs")
            nc.scalar.copy(gT[:], gT_ps[:])
            o_ps = psT.tile([P, out_f], dt.float32, tag="ops")
            nc.tensor.matmul(o_ps[:], lhsT=gT[:], rhs=WlT_sb[:],
                             start=True, stop=True)
            o_sb = tlp.tile([P, out_f], dt.float32, tag="osb")
            nc.scalar.copy(o_sb[:], o_ps[:])
            nc.sync.dma_start(out[b * P:(b + 1) * P, :], o_sb[:])
        if loop_ctx is not None:
            loop_ctx.__exit__(None, None, None)

    nc.compile()
    return nc


# ---------------------------------------------------------------- entry point

N_NODES, N_EDGES, IN_F, HID, OUT_F = 50000, 800000, 256, 128, 64
N_CORES = 8

_cache = {}


def kernel(x, src, dst, W_gat, attn_l, attn_r, bias_gat, W_lin):
    """Full-input GAT layer on 8 NeuronCores; returns [N_NODES, OUT_F] fp32."""
    from concourse.bass_utils import run_bass_kernel_spmd

    src = np.asarray(src)
    dst = np.asarray(dst)
    key = (src.tobytes(), dst.tobytes())
    ck = _cache.get("k")
    if ck is not None and ck[0] == key:
        sched, nc = ck[1], ck[2]
    else:
        sched, per_core = preprocess(src, dst, N_NODES, N_CORES)
        _cache["pc"] = per_core
        nc = build_program(sched, IN_F, HID, OUT_F)
        _cache["k"] = (key, sched, nc)
        ck = _cache["k"]
    sched = ck[1]
    per_core = _cache["pc"]
    in_maps = make_core_inputs(sched, per_core, x, W_gat, attn_l, attn_r,
                               bias_gat, W_lin)
    res = run_bass_kernel_spmd(nc, in_maps, core_ids=list(range(N_CORES)))
    out = np.concatenate(
        [res.results[c]["out"][:sched["npc"]] for c in range(N_CORES)], axis=0)
    return out.astype(np.float32)


# revision 19
# speedup vs baseline: 1.2836x; 1.0737x over previous
"""GAT message-passing kernel for TRN2: host preprocessing + Bass/Tile program.

v4 "alpha-field" design (per core, SPMD over 8 cores, dst-sharded nodes):
  phase 0: EVERY core computes the FULL feat table (x @ W_gat, bf16) from a
           host-rotated copy of x so its own dst shard occupies blocks 0..48.
           No collective. er = feat . attn_r for own blocks -> er_flat row.
  edge phase, per dst block of 128 nodes (edges pre-sorted by (dst blk, src)):
    - erb = ones x er_row rank-1 matmul  (PSUM [P,P]: er[d] bcast down parts)
    - dma_gather feat rows of the block's edge sources (bf16 256B rows,
      lo/hi half-table gathers for int16 indices)   <- gpsimd critical path
    - el (per edge) batched: t = G*attn_l (bf16), el = reduce_X(t)  [2 DVE]
    - per 128-edge chunk j:
        lz  = ACT Lrelu(erb + el_col_j)      (alpha-field, [P,128] bf16)
        alf = ACT Exp(lz)
        Oa  = DVE stt: (iota == dst_j) * alf (one DVE op per chunk)
        pA += Oa^T @ G_j      (agg unnorm, PSUM [128 dst, hid])
        pB += Oa^T @ ones_col (denom,      PSUM [128 dst, 1])
    - tail: rec = 1/max(pB,eps) [P,1]; gene = pA*rec + bias;
      geneL = ACT Lrelu(0.01) bf16; out = (geneL^T)^T @ W_lin^T via PE
      transpose + bf16 matmul.
Softmax max-subtraction dropped (exp args bounded ~ +-13; ratios identical).
"""

import numpy as np
import ml_dtypes
from contextlib import ExitStack

import concourse.bass as bass
import concourse.tile as tile
from concourse import bacc, mybir
from concourse import library_config

dt = mybir.dt
P = 128
PAD_DST = 512.0  # one-hot miss sentinel (exact in bf16, > 127)


# ---------------------------------------------------------------- host side

def preprocess(src, dst, n_nodes, n_cores):
    """Pure index-space preprocessing (no float math on values).

    Per-core node rotation: core c's table is the global padded table rotated
    by c*npc_pad so that its own dst shard occupies rows [0, npc_pad).
    """
    src = np.asarray(src).astype(np.int64)
    dst = np.asarray(dst).astype(np.int64)
    npc = n_nodes // n_cores                      # nodes per core
    assert npc * n_cores == n_nodes
    blocks = (npc + P - 1) // P
    npc_pad = blocks * P                          # padded nodes per core
    n_pad = npc_pad * n_cores                     # padded global node count
    half = n_pad // 2                             # low table rows [0, half)
    assert half <= 32767 and (n_pad - half) <= 32767

    core_of = dst // npc
    blk_of = (dst % npc) // P
    dloc_of = (dst % npc) % P
    srcp = (src // npc) * npc_pad + (src % npc)   # padded source coordinates

    order = np.lexsort((srcp, blk_of, core_of))
    s_s, c_s, b_s, d_s = srcp[order], core_of[order], blk_of[order], dloc_of[order]

    # per-core rotated source ids and lo/hi bucketing
    lo_lists = [[None] * blocks for _ in range(n_cores)]
    hi_lists = [[None] * blocks for _ in range(n_cores)]
    for c in range(n_cores):
        cm = c_s == c
        s_rot = (s_s - c * npc_pad) % n_pad
        hi_mask = s_rot >= half
        for b in range(blocks):
            m = cm & (b_s == b)
            ml = m & ~hi_mask
            mh = m & hi_mask
            ol = np.argsort(s_rot[ml], kind="stable")
            oh = np.argsort(s_rot[mh], kind="stable")
            lo_lists[c][b] = (s_rot[ml][ol], d_s[ml][ol])
            hi_lists[c][b] = (s_rot[mh][oh] - half, d_s[mh][oh])

    def nchunks(n):
        return (n + P - 1) // P

    C_lo = [max(max(nchunks(len(lo_lists[c][b][0])) for c in range(n_cores)), 1)
            for b in range(blocks)]
    C_hi = [max(nchunks(len(hi_lists[c][b][0])) for c in range(n_cores))
            for b in range(blocks)]

    total_chunks = sum(C_lo) + sum(C_hi)
    total_L = total_chunks * P

    per_core = []
    for c in range(n_cores):
        idx = np.zeros(total_L, dtype=np.int16)
        dstf = np.full(total_L, PAD_DST, dtype=np.float32)
        off = 0
        for b in range(blocks):
            for lists, C in ((lo_lists, C_lo[b]), (hi_lists, C_hi[b])):
                L = C * P
                if L == 0:
                    continue
                s_arr, d_arr = lists[c][b]
                n = len(s_arr)
                idx[off:off + n] = s_arr.astype(np.int16)
                # pads keep idx 0 (real row; killed by dst sentinel)
                dstf[off:off + n] = d_arr.astype(np.float32)
                off += L
        assert off == total_L
        # wrapped int16 layout: index i lives at [i % 16, i // 16];
        # replicated 8x down partitions (one copy per Q7 core)
        idx16 = np.tile(idx.reshape(total_L // 16, 16).T, (8, 1)).copy()
        # additive alpha-field mask: 0 at (e, dst_e), -1000 elsewhere (leaky
        # scales negatives by 0.2 -> exp(0.2*(z-1000)) underflows to 0); pads all -1000
        dloc = dstf.reshape(total_chunks, P).astype(np.int64)  # [g, e]
        M3 = np.full((total_chunks, P, P), -1000.0, dtype=ml_dtypes.bfloat16)
        gg, ee = np.nonzero(dloc < P)
        M3[gg, ee, dloc[gg, ee]] = 0.0
        Mmask = np.ascontiguousarray(
            M3.transpose(1, 0, 2).reshape(P, total_chunks * P))
        per_core.append({"idx16": idx16, "Mmask": Mmask})

    sched = {
        "n_nodes": n_nodes, "n_cores": n_cores, "npc": npc, "blocks": blocks,
        "npc_pad": npc_pad, "n_pad": n_pad,
        "half": half, "C_lo": C_lo, "C_hi": C_hi,
        "total_chunks": total_chunks, "total_L": total_L,
        "CMAX": max(C_lo[b] + C_hi[b] for b in range(blocks)),
    }
    return sched, per_core


def make_core_inputs(sched, per_core, x, W_gat, attn_l, attn_r, bias_gat, W_lin):
    """Per-core in_maps. Only permutation/replication/padding/dtype-cast."""
    n_cores, npc, blocks = sched["n_cores"], sched["npc"], sched["blocks"]
    npc_pad, n_pad = sched["npc_pad"], sched["n_pad"]
    in_f = x.shape[1]
    hid = W_gat.shape[1]
    bf = ml_dtypes.bfloat16
    x = np.asarray(x, dtype=np.float32)
    # padded node-major x (global): row (c*npc_pad + i) = x[c*npc + i]
    xpad = np.zeros((n_pad, in_f), dtype=np.float32)
    for c in range(n_cores):
        xpad[c * npc_pad:c * npc_pad + npc] = x[c * npc:(c + 1) * npc]
    Wg_bf = np.asarray(W_gat, np.float32).astype(bf)
    Wg_ext = np.zeros((in_f, hid + 2), dtype=bf)
    Wg_ext[:, 0:hid] = Wg_bf
    WgT_bf = np.ascontiguousarray(np.asarray(W_gat, np.float32).T).astype(bf)
    al_col = np.asarray(attn_l, np.float32).astype(bf)[:, None].copy()
    attnr_b = np.broadcast_to(np.asarray(attn_r, np.float32), (P, hid)).copy()
    bias_b = np.broadcast_to(np.asarray(bias_gat, np.float32), (P, hid)).copy()
    WlT_bf = np.ascontiguousarray(np.asarray(W_lin, np.float32).T).astype(bf)
    ident_bf = np.eye(P, dtype=bf)
    in_maps = []
    for c in range(n_cores):
        # rotate so core c's shard is first
        xrot = np.roll(xpad, -c * npc_pad, axis=0)
        m = {
            "xT": np.ascontiguousarray(xrot.T).astype(bf),  # [in_f, n_pad]
            "Wg": Wg_ext,
            "WgT": WgT_bf,
            "al_col": al_col,
            "attnr_b": attnr_b,
            "bias_b": bias_b,
            "WlT": WlT_bf,
            "ident_bf": ident_bf,
            "idx16": per_core[c]["idx16"],
            "Mmask": per_core[c]["Mmask"],
        }
        in_maps.append(m)
    return in_maps


# ---------------------------------------------------------------- device side

def build_program(sched, in_f, hid, out_f, attn_slope=0.2, act_slope=0.01,
                  n_repeat=1):
    n_cores = sched["n_cores"]
    blocks, half, n_pad = sched["blocks"], sched["half"], sched["n_pad"]
    C_lo, C_hi = sched["C_lo"], sched["C_hi"]
    total_chunks, total_L = sched["total_chunks"], sched["total_L"]
    CMAX = sched["CMAX"]
    nblk_all = n_pad // P
    assert in_f % P == 0 and hid == P
    KT = in_f // P

    nc = bacc.Bacc("TRN2", target_bir_lowering=False, debug=False,
                   num_devices=n_cores, num_swdge_queues=4)

    def din(name, shape, dtype):
        return nc.dram_tensor(name, shape, dtype, kind="ExternalInput").ap()

    HE = 256  # extended table row: [feat(128) | 1.0 | el | junk]
    xT = din("xT", [in_f, n_pad], dt.bfloat16)
    Wg = din("Wg", [in_f, hid + 2], dt.bfloat16)
    WgT = din("WgT", [hid, in_f], dt.bfloat16)
    al_col_in = din("al_col", [hid, 1], dt.bfloat16)
    attnr_b = din("attnr_b", [P, hid], dt.float32)
    bias_b = din("bias_b", [P, hid], dt.float32)
    WlT = din("WlT", [hid, out_f], dt.bfloat16)
    ident_bf = din("ident_bf", [P, P], dt.bfloat16)
    idx16 = din("idx16", [128, total_L // 16], dt.int16)
    Mmask = din("Mmask", [P, total_L], dt.bfloat16)
    out = nc.dram_tensor("out", [blocks * P, out_f], dt.float32,
                         kind="ExternalOutput").ap()

    table_lo = nc.dram_tensor("table_lo", [half, HE], dt.bfloat16).ap()
    table_hi = nc.dram_tensor("table_hi", [n_pad - half, HE], dt.bfloat16).ap()
    er_lin = nc.dram_tensor("er_lin", [blocks * P], dt.bfloat16).ap()

    BW = 7  # phase-0 node blocks per DMA batch (aligns half=28*BW*P)
    assert nblk_all % BW == 0

    with ExitStack() as ctx:
        tc = ctx.enter_context(tile.TileContext(nc))
        nc.gpsimd.load_library(library_config.mlp)
        const = ctx.enter_context(tc.tile_pool(name="const", bufs=1))

        # ---- constants in SBUF
        ident_sb = const.tile([P, P], dt.bfloat16)
        nc.sync.dma_start(ident_sb[:], ident_bf[:])
        attnr_sb = const.tile([P, hid], dt.float32)
        nc.sync.dma_start(attnr_sb[:], attnr_b[:])
        bias_sb = const.tile([P, hid], dt.float32)
        nc.sync.dma_start(bias_sb[:], bias_b[:])
        WlT_sb = const.tile([hid, out_f], dt.bfloat16)
        nc.sync.dma_start(WlT_sb[:], WlT[:])
        WgT_sb = const.tile([hid, in_f], dt.bfloat16)
        nc.sync.dma_start(WgT_sb[:], WgT[:])
        al_col_sb = const.tile([hid, 1], dt.bfloat16)
        nc.sync.dma_start(al_col_sb[:], al_col_in[:])
        Wg_sb = []
        for k in range(KT):
            w = const.tile([P, hid + 2], dt.bfloat16, tag=f"Wg{k}")
            nc.sync.dma_start(w[:, 0:hid + 2], Wg[k * P:(k + 1) * P, :])
            Wg_sb.append(w)
        al02 = const.tile([P, 1], dt.float32)
        nc.vector.memset(al02[:], 0.2)
        al001 = const.tile([P, 1], dt.float32)
        nc.vector.memset(al001[:], 0.01)
        ones_row = const.tile([1, P], dt.bfloat16)
        nc.vector.memset(ones_row[:], 1.0)
        ones_col = const.tile([P, 1], dt.bfloat16)
        nc.vector.memset(ones_col[:], 1.0)
        idx_sb = const.tile([128, total_L // 16], dt.int16)
        nc.sync.dma_start(idx_sb[:], idx16[:])
        er_sb = const.tile([1, blocks * P], dt.bfloat16)

        # ---- phase 0: full feat table + er for own blocks
        xp = ctx.enter_context(tc.tile_pool(name="xp", bufs=6))
        ps0 = ctx.enter_context(tc.tile_pool(name="ps0", bufs=3, space="PSUM"))
        fbp = ctx.enter_context(tc.tile_pool(name="fbp", bufs=4))
        erp = ctx.enter_context(tc.tile_pool(name="erp", bufs=2))

        # v = Wg @ attn_l -> Wg_ext column hid+1 (el producer)
        for k in range(KT):
            psV = ps0.tile([P, 1], dt.float32, tag="fp")
            nc.tensor.matmul(psV[:], lhsT=WgT_sb[:, k * P:(k + 1) * P],
                             rhs=al_col_sb[:], start=True, stop=True)
            nc.scalar.copy(Wg_sb[k][:, hid:hid + 1], psV[:])
        assert half % (BW * P) == 0
        for g4 in range(nblk_all // BW):
            r0 = g4 * BW * P
            tbl = table_lo if r0 < half else table_hi
            toff = r0 if r0 < half else r0 - half
            xa = xp.tile([P, KT * BW * P], dt.bfloat16, tag="xa")
            nc.scalar.dma_start(
                xa[:].rearrange("p (k n) -> p k n", n=BW * P),
                xT[:, r0:r0 + BW * P].rearrange("(k p) n -> p k n", k=KT))
            fb4 = fbp.tile([P, BW * HE], dt.bfloat16, tag="fb4")
            nc.vector.memset(
                fb4[:].rearrange("p (i h) -> p i h", h=HE)[:, :,
                                                           hid + 1:hid + 2],
                1.0)
            for i in range(BW):
                nb = g4 * BW + i
                fp = ps0.tile([P, hid + 2], dt.float32, tag="fp")
                for k in range(KT):
                    nc.tensor.matmul(
                        fp[:], lhsT=xa[:, k * BW * P + i * P:
                                       k * BW * P + (i + 1) * P],
                        rhs=Wg_sb[k][:], start=(k == 0),
                        stop=(k == KT - 1))
                if i % 2 == 0:
                    nc.scalar.copy(fb4[:, i * HE:i * HE + hid + 1],
                                   fp[:, 0:hid + 1])
                else:
                    nc.vector.tensor_copy(fb4[:, i * HE:i * HE + hid + 1],
                                          fp[:, 0:hid + 1])
                if nb < blocks:
                    er_f = erp.tile([P, 1], dt.float32, tag="erf")
                    scr = erp.tile([P, hid], dt.float32, tag="scr")
                    nc.vector.scalar_tensor_tensor(
                        out=scr[:], in0=fp[:, 0:hid], scalar=1.0,
                        in1=attnr_sb[:],
                        op0=mybir.AluOpType.bypass, op1=mybir.AluOpType.mult,
                        accum_out=er_f[:])
                    er_b = erp.tile([P, 1], dt.bfloat16, tag="erb16")
                    nc.scalar.copy(er_b[:], er_f[:])
                    nc.sync.dma_start(er_lin[nb * P:(nb + 1) * P, None],
                                       er_b[:])
            nc.sync.dma_start(
                tbl[toff:toff + BW * P, :].rearrange(
                    "(i p) h -> p i h", i=BW),
                fb4[:].rearrange("p (i h) -> p i h", h=HE))
        nc.scalar.dma_start(er_sb[:], er_lin[:][None, :])

        # ---- edge phase
        gp = ctx.enter_context(tc.tile_pool(name="gp", bufs=4))
        mp = ctx.enter_context(tc.tile_pool(name="mp", bufs=4))
        ebp = ctx.enter_context(tc.tile_pool(name="ebp", bufs=2))
        zp = ctx.enter_context(tc.tile_pool(name="zp", bufs=2))
        alp = ctx.enter_context(tc.tile_pool(name="alp", bufs=3))
        tlp = ctx.enter_context(tc.tile_pool(name="tlp", bufs=2))
        psErb = ctx.enter_context(tc.tile_pool(name="psErb", bufs=1, space="PSUM"))
        psA = ctx.enter_context(tc.tile_pool(name="psA", bufs=2, space="PSUM"))
        psT = ctx.enter_context(tc.tile_pool(name="psT", bufs=1, space="PSUM"))

        _gq = [0]
        loop_ctx = tc.For_i(0, n_repeat, 1) if n_repeat > 1 else None
        if loop_ctx is not None:
            loop_ctx.__enter__()
        if True:
          g = 0
          for b in range(blocks):
            Cl, Ch = C_lo[b], C_hi[b]
            C = Cl + Ch
            erb_ps = psErb.tile([P, P], dt.float32, tag="erb")
            nc.tensor.matmul(erb_ps[:], lhsT=ones_row[:],
                             rhs=er_sb[:, b * P:(b + 1) * P],
                             start=True, stop=True)
            erb_sb = ebp.tile([P, P], dt.bfloat16, tag="erbs")
            nc.scalar.copy(erb_sb[:], erb_ps[:])
            Mt = mp.tile([P, C * P], dt.bfloat16, tag="Mt")
            nc.scalar.dma_start(Mt[:], Mmask[:, g * P:(g + C) * P])

            G = gp.tile([P, C * HE], dt.bfloat16, tag="G")
            G3 = G[:].rearrange("p (c h) -> p c h", h=HE)
            o16 = (g * P) // 16
            GMAX = 6  # chunks per dma_gather; 768 idxs < 1024-desc SWDGE ring
            for cbase, ccnt, tbl in (
                    [(c0, min(GMAX, Cl - c0), table_lo)
                     for c0 in range(0, Cl, GMAX)] +
                    [(Cl + c0, min(GMAX, Ch - c0), table_hi)
                     for c0 in range(0, Ch, GMAX)]):
                nc.gpsimd.dma_gather(
                    G3[:, cbase:cbase + ccnt, :], tbl,
                    idx_sb[:, o16 + cbase * 8:o16 + (cbase + ccnt) * 8],
                    ccnt * P, ccnt * P, HE, elem_step=HE,
                    queue_num=_gq[0] % 4)
                _gq[0] += 1

            z1 = zp.tile([P, C * P], dt.bfloat16, tag="z1")
            nc.vector.scalar_tensor_tensor(
                out=z1[:].rearrange("p (c f) -> p c f", f=P),
                in0=Mt[:].rearrange("p (c f) -> p c f", f=P),
                scalar=1.0,
                in1=erb_sb[:].unsqueeze(1).broadcast_to([P, C, P]),
                op0=mybir.AluOpType.bypass, op1=mybir.AluOpType.add)
            z2 = zp.tile([P, C * P], dt.bfloat16, tag="z2")
            nc.vector.scalar_tensor_tensor(
                out=z2[:].rearrange("p (c f) -> p c f", f=P),
                in0=z1[:].rearrange("p (c f) -> p c f", f=P),
                scalar=1.0,
                in1=G3[:, :, hid:hid + 1].broadcast_to([P, C, P]),
                op0=mybir.AluOpType.bypass, op1=mybir.AluOpType.add)
            lzt = zp.tile([P, C * P], dt.bfloat16, tag="lz")
            nc.vector.scalar_tensor_tensor(
                out=lzt[:], in0=z2[:], scalar=float(attn_slope),
                in1=z2[:], op0=mybir.AluOpType.mult, op1=mybir.AluOpType.max)
            alf = alp.tile([P, C * P], dt.bfloat16, tag="alf")
            nc.scalar.activation(alf[:], lzt[:],
                                 mybir.ActivationFunctionType.Exp)
            pAB = psA.tile([P, hid + 2], dt.float32, tag="psA")
            pA = pAB[:, 0:hid]
            pB = pAB[:, hid + 1:hid + 2]
            for j in range(C):
                nc.tensor.matmul(pAB[:], lhsT=alf[:, j * P:(j + 1) * P],
                                 rhs=G3[:, j, 0:hid + 2],
                                 start=(j == 0), stop=(j == C - 1))
            g += C

            den = tlp.tile([P, 1], dt.float32, tag="den")
            nc.vector.tensor_scalar(den[:], pB, 1e-30, None,
                                    mybir.AluOpType.max)
            rec = tlp.tile([P, 1], dt.float32, tag="rec")
            nc.vector.reciprocal(rec[:], den[:])
            gene = tlp.tile([P, hid], dt.float32, tag="gene")
            nc.vector.scalar_tensor_tensor(
                out=gene[:], in0=pA, scalar=rec[:], in1=bias_sb[:],
                op0=mybir.AluOpType.mult, op1=mybir.AluOpType.add)
            geneL = tlp.tile([P, hid], dt.bfloat16, tag="geneL")
            nc.scalar.activation(geneL[:], gene[:],
                                 mybir.ActivationFunctionType.Prelu,
                                 alpha=al001[:])
            gT_ps = psT.tile([hid, P], dt.bfloat16, tag="gT")
            nc.tensor.transpose(gT_ps[:], geneL[:], ident_sb[:])
            gT = tlp.tile([hid, P], dt.bfloat16, tag="gTs")
            nc.scalar.copy(gT[:], gT_ps[:])
            o_ps = psT.tile([P, out_f], dt.float32, tag="ops")
            nc.tensor.matmul(o_ps[:], lhsT=gT[:], rhs=WlT_sb[:],
                             start=True, stop=True)
            o_sb = tlp.tile([P, out_f], dt.float32, tag="osb")
            nc.scalar.copy(o_sb[:], o_ps[:])
            nc.scalar.dma_start(out[b * P:(b + 1) * P, :], o_sb[:])
        if loop_ctx is not None:
            loop_ctx.__exit__(None, None, None)

    nc.compile()
    return nc


# ---------------------------------------------------------------- entry point

N_NODES, N_EDGES, IN_F, HID, OUT_F = 50000, 800000, 256, 128, 64
N_CORES = 8

_cache = {}


def kernel(x, src, dst, W_gat, attn_l, attn_r, bias_gat, W_lin):
    """Full-input GAT layer on 8 NeuronCores; returns [N_NODES, OUT_F] fp32."""
    from concourse.bass_utils import run_bass_kernel_spmd

    src = np.asarray(src)
    dst = np.asarray(dst)
    key = (src.tobytes(), dst.tobytes())
    ck = _cache.get("k")
    if ck is not None and ck[0] == key:
        sched, nc = ck[1], ck[2]
    else:
        sched, per_core = preprocess(src, dst, N_NODES, N_CORES)
        _cache["pc"] = per_core
        nc = build_program(sched, IN_F, HID, OUT_F)
        _cache["k"] = (key, sched, nc)
        ck = _cache["k"]
    sched = ck[1]
    per_core = _cache["pc"]
    in_maps = make_core_inputs(sched, per_core, x, W_gat, attn_l, attn_r,
                               bias_gat, W_lin)
    res = run_bass_kernel_spmd(nc, in_maps, core_ids=list(range(N_CORES)))
    out = np.concatenate(
        [res.results[c]["out"][:sched["npc"]] for c in range(N_CORES)], axis=0)
    return out.astype(np.float32)


# revision 20
# speedup vs baseline: 1.3067x; 1.0180x over previous
"""GAT message-passing kernel for TRN2: host preprocessing + Bass/Tile program.

v4 "alpha-field" design (per core, SPMD over 8 cores, dst-sharded nodes):
  phase 0: EVERY core computes the FULL feat table (x @ W_gat, bf16) from a
           host-rotated copy of x so its own dst shard occupies blocks 0..48.
           No collective. er = feat . attn_r for own blocks -> er_flat row.
  edge phase, per dst block of 128 nodes (edges pre-sorted by (dst blk, src)):
    - erb = ones x er_row rank-1 matmul  (PSUM [P,P]: er[d] bcast down parts)
    - dma_gather feat rows of the block's edge sources (bf16 256B rows,
      lo/hi half-table gathers for int16 indices)   <- gpsimd critical path
    - el (per edge) batched: t = G*attn_l (bf16), el = reduce_X(t)  [2 DVE]
    - per 128-edge chunk j:
        lz  = ACT Lrelu(erb + el_col_j)      (alpha-field, [P,128] bf16)
        alf = ACT Exp(lz)
        Oa  = DVE stt: (iota == dst_j) * alf (one DVE op per chunk)
        pA += Oa^T @ G_j      (agg unnorm, PSUM [128 dst, hid])
        pB += Oa^T @ ones_col (denom,      PSUM [128 dst, 1])
    - tail: rec = 1/max(pB,eps) [P,1]; gene = pA*rec + bias;
      geneL = ACT Lrelu(0.01) bf16; out = (geneL^T)^T @ W_lin^T via PE
      transpose + bf16 matmul.
Softmax max-subtraction dropped (exp args bounded ~ +-13; ratios identical).
"""

import numpy as np
import ml_dtypes
from contextlib import ExitStack

import concourse.bass as bass
import concourse.tile as tile
from concourse import bacc, mybir
from concourse import library_config

dt = mybir.dt
P = 128
PAD_DST = 512.0  # one-hot miss sentinel (exact in bf16, > 127)


# ---------------------------------------------------------------- host side

def preprocess(src, dst, n_nodes, n_cores):
    """Pure index-space preprocessing (no float math on values).

    Per-core node rotation: core c's table is the global padded table rotated
    by c*npc_pad so that its own dst shard occupies rows [0, npc_pad).
    """
    src = np.asarray(src).astype(np.int64)
    dst = np.asarray(dst).astype(np.int64)
    npc = n_nodes // n_cores                      # nodes per core
    assert npc * n_cores == n_nodes
    blocks = (npc + P - 1) // P
    npc_pad = blocks * P                          # padded nodes per core
    n_pad = npc_pad * n_cores                     # padded global node count
    half = n_pad // 2                             # low table rows [0, half)
    assert half <= 32767 and (n_pad - half) <= 32767

    core_of = dst // npc
    blk_of = (dst % npc) // P
    dloc_of = (dst % npc) % P
    srcp = (src // npc) * npc_pad + (src % npc)   # padded source coordinates

    order = np.lexsort((srcp, blk_of, core_of))
    s_s, c_s, b_s, d_s = srcp[order], core_of[order], blk_of[order], dloc_of[order]

    # per-core rotated source ids and lo/hi bucketing
    lo_lists = [[None] * blocks for _ in range(n_cores)]
    hi_lists = [[None] * blocks for _ in range(n_cores)]
    nblk_all = n_pad // P
    for c in range(n_cores):
        cm = c_s == c
        s_rot = (s_s - c * npc_pad) % n_pad
        # interleaved table id: row (n % P) * nblk_all + (n // P); the lo/hi
        # halves are partitions [0,64) / [64,128) exactly (half = 64*nblk_all)
        idp = (s_rot % P) * nblk_all + (s_rot // P)
        hi_mask = idp >= half
        for b in range(blocks):
            m = cm & (b_s == b)
            ml = m & ~hi_mask
            mh = m & hi_mask
            ol = np.argsort(idp[ml], kind="stable")
            oh = np.argsort(idp[mh], kind="stable")
            lo_lists[c][b] = (idp[ml][ol], d_s[ml][ol])
            hi_lists[c][b] = (idp[mh][oh] - half, d_s[mh][oh])

    def nchunks(n):
        return (n + P - 1) // P

    C_lo = [max(max(nchunks(len(lo_lists[c][b][0])) for c in range(n_cores)), 1)
            for b in range(blocks)]
    C_hi = [max(nchunks(len(hi_lists[c][b][0])) for c in range(n_cores))
            for b in range(blocks)]

    total_chunks = sum(C_lo) + sum(C_hi)
    total_L = total_chunks * P

    per_core = []
    for c in range(n_cores):
        idx = np.zeros(total_L, dtype=np.int16)
        dstf = np.full(total_L, PAD_DST, dtype=np.float32)
        off = 0
        for b in range(blocks):
            for lists, C in ((lo_lists, C_lo[b]), (hi_lists, C_hi[b])):
                L = C * P
                if L == 0:
                    continue
                s_arr, d_arr = lists[c][b]
                n = len(s_arr)
                idx[off:off + n] = s_arr.astype(np.int16)
                # pads keep idx 0 (real row; killed by dst sentinel)
                dstf[off:off + n] = d_arr.astype(np.float32)
                off += L
        assert off == total_L
        # wrapped int16 layout: index i lives at [i % 16, i // 16];
        # replicated 8x down partitions (one copy per Q7 core)
        idx16 = np.tile(idx.reshape(total_L // 16, 16).T, (8, 1)).copy()
        # additive alpha-field mask: 0 at (e, dst_e), -1000 elsewhere (leaky
        # scales negatives by 0.2 -> exp(0.2*(z-1000)) underflows to 0); pads all -1000
        dloc = dstf.reshape(total_chunks, P).astype(np.int64)  # [g, e]
        M3 = np.full((total_chunks, P, P), -1000.0, dtype=ml_dtypes.bfloat16)
        gg, ee = np.nonzero(dloc < P)
        M3[gg, ee, dloc[gg, ee]] = 0.0
        Mmask = np.ascontiguousarray(
            M3.transpose(1, 0, 2).reshape(P, total_chunks * P))
        per_core.append({"idx16": idx16, "Mmask": Mmask})

    sched = {
        "n_nodes": n_nodes, "n_cores": n_cores, "npc": npc, "blocks": blocks,
        "npc_pad": npc_pad, "n_pad": n_pad,
        "half": half, "C_lo": C_lo, "C_hi": C_hi,
        "total_chunks": total_chunks, "total_L": total_L,
        "CMAX": max(C_lo[b] + C_hi[b] for b in range(blocks)),
    }
    return sched, per_core


def make_core_inputs(sched, per_core, x, W_gat, attn_l, attn_r, bias_gat, W_lin):
    """Per-core in_maps. Only permutation/replication/padding/dtype-cast."""
    n_cores, npc, blocks = sched["n_cores"], sched["npc"], sched["blocks"]
    npc_pad, n_pad = sched["npc_pad"], sched["n_pad"]
    in_f = x.shape[1]
    hid = W_gat.shape[1]
    bf = ml_dtypes.bfloat16
    x = np.asarray(x, dtype=np.float32)
    # padded node-major x (global): row (c*npc_pad + i) = x[c*npc + i]
    xpad = np.zeros((n_pad, in_f), dtype=np.float32)
    for c in range(n_cores):
        xpad[c * npc_pad:c * npc_pad + npc] = x[c * npc:(c + 1) * npc]
    Wg_bf = np.asarray(W_gat, np.float32).astype(bf)
    Wg_ext = np.zeros((in_f, hid + 2), dtype=bf)
    Wg_ext[:, 0:hid] = Wg_bf
    WgT_bf = np.ascontiguousarray(np.asarray(W_gat, np.float32).T).astype(bf)
    al_col = np.asarray(attn_l, np.float32).astype(bf)[:, None].copy()
    attnr_b = np.broadcast_to(np.asarray(attn_r, np.float32), (P, hid)).copy()
    bias_b = np.broadcast_to(np.asarray(bias_gat, np.float32), (P, hid)).copy()
    WlT_bf = np.ascontiguousarray(np.asarray(W_lin, np.float32).T).astype(bf)
    ident_bf = np.eye(P, dtype=bf)
    in_maps = []
    for c in range(n_cores):
        # rotate so core c's shard is first
        xrot = np.roll(xpad, -c * npc_pad, axis=0)
        m = {
            "xT": np.ascontiguousarray(xrot.T).astype(bf),  # [in_f, n_pad]
            "Wg": Wg_ext,
            "WgT": WgT_bf,
            "al_col": al_col,
            "attnr_b": attnr_b,
            "bias_b": bias_b,
            "WlT": WlT_bf,
            "ident_bf": ident_bf,
            "idx16": per_core[c]["idx16"],
            "Mmask": per_core[c]["Mmask"],
        }
        in_maps.append(m)
    return in_maps


# ---------------------------------------------------------------- device side

def build_program(sched, in_f, hid, out_f, attn_slope=0.2, act_slope=0.01,
                  n_repeat=1):
    n_cores = sched["n_cores"]
    blocks, half, n_pad = sched["blocks"], sched["half"], sched["n_pad"]
    C_lo, C_hi = sched["C_lo"], sched["C_hi"]
    total_chunks, total_L = sched["total_chunks"], sched["total_L"]
    CMAX = sched["CMAX"]
    nblk_all = n_pad // P
    assert in_f % P == 0 and hid == P
    KT = in_f // P

    nc = bacc.Bacc("TRN2", target_bir_lowering=False, debug=False,
                   num_devices=n_cores, num_swdge_queues=4)

    def din(name, shape, dtype):
        return nc.dram_tensor(name, shape, dtype, kind="ExternalInput").ap()

    HE = 256  # extended table row: [feat(128) | 1.0 | el | junk]
    xT = din("xT", [in_f, n_pad], dt.bfloat16)
    Wg = din("Wg", [in_f, hid + 2], dt.bfloat16)
    WgT = din("WgT", [hid, in_f], dt.bfloat16)
    al_col_in = din("al_col", [hid, 1], dt.bfloat16)
    attnr_b = din("attnr_b", [P, hid], dt.float32)
    bias_b = din("bias_b", [P, hid], dt.float32)
    WlT = din("WlT", [hid, out_f], dt.bfloat16)
    ident_bf = din("ident_bf", [P, P], dt.bfloat16)
    idx16 = din("idx16", [128, total_L // 16], dt.int16)
    Mmask = din("Mmask", [P, total_L], dt.bfloat16)
    out = nc.dram_tensor("out", [blocks * P, out_f], dt.float32,
                         kind="ExternalOutput").ap()

    table_lo = nc.dram_tensor("table_lo", [half, HE], dt.bfloat16).ap()
    table_hi = nc.dram_tensor("table_hi", [n_pad - half, HE], dt.bfloat16).ap()
    er_lin = nc.dram_tensor("er_lin", [blocks * P], dt.bfloat16).ap()

    BW = 7  # phase-0 node blocks per DMA batch (aligns half=28*BW*P)
    assert nblk_all % BW == 0

    with ExitStack() as ctx:
        tc = ctx.enter_context(tile.TileContext(nc))
        nc.gpsimd.load_library(library_config.mlp)
        const = ctx.enter_context(tc.tile_pool(name="const", bufs=1))

        # ---- constants in SBUF
        ident_sb = const.tile([P, P], dt.bfloat16)
        nc.sync.dma_start(ident_sb[:], ident_bf[:])
        attnr_sb = const.tile([P, hid], dt.float32)
        nc.sync.dma_start(attnr_sb[:], attnr_b[:])
        bias_sb = const.tile([P, hid], dt.float32)
        nc.sync.dma_start(bias_sb[:], bias_b[:])
        WlT_sb = const.tile([hid, out_f], dt.bfloat16)
        nc.sync.dma_start(WlT_sb[:], WlT[:])
        WgT_sb = const.tile([hid, in_f], dt.bfloat16)
        nc.sync.dma_start(WgT_sb[:], WgT[:])
        al_col_sb = const.tile([hid, 1], dt.bfloat16)
        nc.sync.dma_start(al_col_sb[:], al_col_in[:])
        Wg_sb = []
        for k in range(KT):
            w = const.tile([P, hid + 2], dt.bfloat16, tag=f"Wg{k}")
            nc.sync.dma_start(w[:, 0:hid + 2], Wg[k * P:(k + 1) * P, :])
            Wg_sb.append(w)
        al02 = const.tile([P, 1], dt.float32)
        nc.vector.memset(al02[:], 0.2)
        al001 = const.tile([P, 1], dt.float32)
        nc.vector.memset(al001[:], 0.01)
        ones_row = const.tile([1, P], dt.bfloat16)
        nc.vector.memset(ones_row[:], 1.0)
        ones_col = const.tile([P, 1], dt.bfloat16)
        nc.vector.memset(ones_col[:], 1.0)
        idx_sb = const.tile([128, total_L // 16], dt.int16)
        nc.sync.dma_start(idx_sb[:], idx16[:])
        er_sb = const.tile([1, blocks * P], dt.bfloat16)

        # ---- phase 0: full feat table + er for own blocks
        xp = ctx.enter_context(tc.tile_pool(name="xp", bufs=6))
        ps0 = ctx.enter_context(tc.tile_pool(name="ps0", bufs=3, space="PSUM"))
        fbp = ctx.enter_context(tc.tile_pool(name="fbp", bufs=4))
        erp = ctx.enter_context(tc.tile_pool(name="erp", bufs=2))

        # v = Wg @ attn_l -> Wg_ext column hid+1 (el producer)
        for k in range(KT):
            psV = ps0.tile([P, 1], dt.float32, tag="fp")
            nc.tensor.matmul(psV[:], lhsT=WgT_sb[:, k * P:(k + 1) * P],
                             rhs=al_col_sb[:], start=True, stop=True)
            nc.scalar.copy(Wg_sb[k][:, hid:hid + 1], psV[:])
        assert half == 64 * nblk_all
        tlo3 = table_lo[:].rearrange("(p i) h -> p i h", i=nblk_all)
        thi3 = table_hi[:].rearrange("(p i) h -> p i h", i=nblk_all)
        for g4 in range(nblk_all // BW):
            r0 = g4 * BW * P
            xa = xp.tile([P, KT * BW * P], dt.bfloat16, tag="xa")
            nc.scalar.dma_start(
                xa[:].rearrange("p (k n) -> p k n", n=BW * P),
                xT[:, r0:r0 + BW * P].rearrange("(k p) n -> p k n", k=KT))
            fb4 = fbp.tile([P, BW * HE], dt.bfloat16, tag="fb4")
            nc.vector.memset(
                fb4[:].rearrange("p (i h) -> p i h", h=HE)[:, :,
                                                           hid + 1:hid + 2],
                1.0)
            for i in range(BW):
                nb = g4 * BW + i
                fp = ps0.tile([P, hid + 2], dt.float32, tag="fp")
                for k in range(KT):
                    nc.tensor.matmul(
                        fp[:], lhsT=xa[:, k * BW * P + i * P:
                                       k * BW * P + (i + 1) * P],
                        rhs=Wg_sb[k][:], start=(k == 0),
                        stop=(k == KT - 1))
                if i % 2 == 0:
                    nc.scalar.copy(fb4[:, i * HE:i * HE + hid + 1],
                                   fp[:, 0:hid + 1])
                else:
                    nc.vector.tensor_copy(fb4[:, i * HE:i * HE + hid + 1],
                                          fp[:, 0:hid + 1])
                if nb < blocks:
                    er_f = erp.tile([P, 1], dt.float32, tag="erf")
                    scr = erp.tile([P, hid], dt.float32, tag="scr")
                    nc.vector.scalar_tensor_tensor(
                        out=scr[:], in0=fp[:, 0:hid], scalar=1.0,
                        in1=attnr_sb[:],
                        op0=mybir.AluOpType.bypass, op1=mybir.AluOpType.mult,
                        accum_out=er_f[:])
                    er_b = erp.tile([P, 1], dt.bfloat16, tag="erb16")
                    nc.scalar.copy(er_b[:], er_f[:])
                    nc.sync.dma_start(er_lin[nb * P:(nb + 1) * P, None],
                                       er_b[:])
            i0 = g4 * BW
            nc.sync.dma_start(
                tlo3[:, i0:i0 + BW, :],
                fb4[0:64, :].rearrange("p (i h) -> p i h", h=HE))
            nc.sync.dma_start(
                thi3[:, i0:i0 + BW, :],
                fb4[64:128, :].rearrange("p (i h) -> p i h", h=HE))
        nc.scalar.dma_start(er_sb[:], er_lin[:][None, :])

        # ---- edge phase
        gp = ctx.enter_context(tc.tile_pool(name="gp", bufs=4))
        mp = ctx.enter_context(tc.tile_pool(name="mp", bufs=4))
        ebp = ctx.enter_context(tc.tile_pool(name="ebp", bufs=2))
        zp = ctx.enter_context(tc.tile_pool(name="zp", bufs=2))
        alp = ctx.enter_context(tc.tile_pool(name="alp", bufs=3))
        tlp = ctx.enter_context(tc.tile_pool(name="tlp", bufs=2))
        psErb = ctx.enter_context(tc.tile_pool(name="psErb", bufs=1, space="PSUM"))
        psA = ctx.enter_context(tc.tile_pool(name="psA", bufs=2, space="PSUM"))
        psT = ctx.enter_context(tc.tile_pool(name="psT", bufs=1, space="PSUM"))

        _gq = [0]
        loop_ctx = tc.For_i(0, n_repeat, 1) if n_repeat > 1 else None
        if loop_ctx is not None:
            loop_ctx.__enter__()
        if True:
          g = 0
          for b in range(blocks):
            Cl, Ch = C_lo[b], C_hi[b]
            C = Cl + Ch
            erb_ps = psErb.tile([P, P], dt.float32, tag="erb")
            nc.tensor.matmul(erb_ps[:], lhsT=ones_row[:],
                             rhs=er_sb[:, b * P:(b + 1) * P],
                             start=True, stop=True)
            erb_sb = ebp.tile([P, P], dt.bfloat16, tag="erbs")
            nc.scalar.copy(erb_sb[:], erb_ps[:])
            Mt = mp.tile([P, C * P], dt.bfloat16, tag="Mt")
            nc.scalar.dma_start(Mt[:], Mmask[:, g * P:(g + C) * P])

            G = gp.tile([P, C * HE], dt.bfloat16, tag="G")
            G3 = G[:].rearrange("p (c h) -> p c h", h=HE)
            o16 = (g * P) // 16
            GMAX = 6  # chunks per dma_gather; 768 idxs < 1024-desc SWDGE ring
            for cbase, ccnt, tbl in (
                    [(c0, min(GMAX, Cl - c0), table_lo)
                     for c0 in range(0, Cl, GMAX)] +
                    [(Cl + c0, min(GMAX, Ch - c0), table_hi)
                     for c0 in range(0, Ch, GMAX)]):
                nc.gpsimd.dma_gather(
                    G3[:, cbase:cbase + ccnt, :], tbl,
                    idx_sb[:, o16 + cbase * 8:o16 + (cbase + ccnt) * 8],
                    ccnt * P, ccnt * P, HE, elem_step=HE,
                    queue_num=_gq[0] % 4)
                _gq[0] += 1

            z1 = zp.tile([P, C * P], dt.bfloat16, tag="z1")
            nc.vector.scalar_tensor_tensor(
                out=z1[:].rearrange("p (c f) -> p c f", f=P),
                in0=Mt[:].rearrange("p (c f) -> p c f", f=P),
                scalar=1.0,
                in1=erb_sb[:].unsqueeze(1).broadcast_to([P, C, P]),
                op0=mybir.AluOpType.bypass, op1=mybir.AluOpType.add)
            z2 = zp.tile([P, C * P], dt.bfloat16, tag="z2")
            nc.vector.scalar_tensor_tensor(
                out=z2[:].rearrange("p (c f) -> p c f", f=P),
                in0=z1[:].rearrange("p (c f) -> p c f", f=P),
                scalar=1.0,
                in1=G3[:, :, hid:hid + 1].broadcast_to([P, C, P]),
                op0=mybir.AluOpType.bypass, op1=mybir.AluOpType.add)
            lzt = zp.tile([P, C * P], dt.bfloat16, tag="lz")
            nc.vector.scalar_tensor_tensor(
                out=lzt[:], in0=z2[:], scalar=float(attn_slope),
                in1=z2[:], op0=mybir.AluOpType.mult, op1=mybir.AluOpType.max)
            alf = alp.tile([P, C * P], dt.bfloat16, tag="alf")
            nc.scalar.activation(alf[:], lzt[:],
                                 mybir.ActivationFunctionType.Exp)
            pAB = psA.tile([P, hid + 2], dt.float32, tag="psA")
            pA = pAB[:, 0:hid]
            pB = pAB[:, hid + 1:hid + 2]
            for j in range(C):
                nc.tensor.matmul(pAB[:], lhsT=alf[:, j * P:(j + 1) * P],
                                 rhs=G3[:, j, 0:hid + 2],
                                 start=(j == 0), stop=(j == C - 1))
            g += C

            den = tlp.tile([P, 1], dt.float32, tag="den")
            nc.vector.tensor_scalar(den[:], pB, 1e-30, None,
                                    mybir.AluOpType.max)
            rec = tlp.tile([P, 1], dt.float32, tag="rec")
            nc.vector.reciprocal(rec[:], den[:])
            gene = tlp.tile([P, hid], dt.float32, tag="gene")
            nc.vector.scalar_tensor_tensor(
                out=gene[:], in0=pA, scalar=rec[:], in1=bias_sb[:],
                op0=mybir.AluOpType.mult, op1=mybir.AluOpType.add)
            geneL = tlp.tile([P, hid], dt.bfloat16, tag="geneL")
            nc.scalar.activation(geneL[:], gene[:],
                                 mybir.ActivationFunctionType.Prelu,
                                 alpha=al001[:])
            gT_ps = psT.tile([hid, P], dt.bfloat16, tag="gT")
            nc.tensor.transpose(gT_ps[:], geneL[:], ident_sb[:])
            gT = tlp.tile([hid, P], dt.bfloat16, tag="gTs")
            nc.scalar.copy(gT[:], gT_ps[:])
            o_ps = psT.tile([P, out_f], dt.float32, tag="ops")
            nc.tensor.matmul(o_ps[:], lhsT=gT[:], rhs=WlT_sb[:],
                             start=True, stop=True)
            o_sb = tlp.tile([P, out_f], dt.float32, tag="osb")
            nc.scalar.copy(o_sb[:], o_ps[:])
            nc.scalar.dma_start(out[b * P:(b + 1) * P, :], o_sb[:])
        if loop_ctx is not None:
            loop_ctx.__exit__(None, None, None)

    nc.compile()
    return nc


# ---------------------------------------------------------------- entry point

N_NODES, N_EDGES, IN_F, HID, OUT_F = 50000, 800000, 256, 128, 64
N_CORES = 8

_cache = {}


def kernel(x, src, dst, W_gat, attn_l, attn_r, bias_gat, W_lin):
    """Full-input GAT layer on 8 NeuronCores; returns [N_NODES, OUT_F] fp32."""
    from concourse.bass_utils import run_bass_kernel_spmd

    src = np.asarray(src)
    dst = np.asarray(dst)
    key = (src.tobytes(), dst.tobytes())
    ck = _cache.get("k")
    if ck is not None and ck[0] == key:
        sched, nc = ck[1], ck[2]
    else:
        sched, per_core = preprocess(src, dst, N_NODES, N_CORES)
        _cache["pc"] = per_core
        nc = build_program(sched, IN_F, HID, OUT_F)
        _cache["k"] = (key, sched, nc)
        ck = _cache["k"]
    sched = ck[1]
    per_core = _cache["pc"]
    in_maps = make_core_inputs(sched, per_core, x, W_gat, attn_l, attn_r,
                               bias_gat, W_lin)
    res = run_bass_kernel_spmd(nc, in_maps, core_ids=list(range(N_CORES)))
    out = np.concatenate(
        [res.results[c]["out"][:sched["npc"]] for c in range(N_CORES)], axis=0)
    return out.astype(np.float32)
